# revision 21
# baseline (speedup 1.0000x reference)
"""Trainium2 Bass kernel for nn_Based_40630390620259 (sparse_attention).

Architecture ("Based"-style): linear (Taylor feature-map) attention +
windowed softmax attention, 16 heads, S=2048, D=1024.

Math identities used (verified against the reference to 1e-6):
  - Taylor feature map inner product collapses:
        qf.kf = 1 + (q.k)/4 + (q.k)^2/32 = 0.5 + 0.5*(1 + q.k/4)^2
    so the 273-dim feature space is never materialized. With Wq,Wk scaled
    by 0.5 on the host and a constant ones-row appended to q/k (K=17
    matmul), the PE produces m'' = 1 + q.k/4 directly; sq = m''^2 on DVE.
  - The 0.5 factor is folded into the V projection weights; the +0.5
    constant term contributes a causal cumulative sum CUM of the
    (0.5-scaled) v rows, computed with 16 N=128 matmuls against an
    upper-triangular ones block + a recursive per-partition scalar-add.
    CUM row 64 (from the 0.5-constant column) equals 0.5*(s+1), which is
    exactly the constant-term part of the reference denominator.
  - win path: scores^T computed as [t,s] tiles; softmax denominator via a
    ones-column in V'; division deferred through the output projection via
    a gpsimd partition_broadcast of the reciprocal row.

Sharding: tensor-parallel over heads, 2 heads per core, 8 cores. Each core
produces a partial [S, D] output (its heads' contribution); the host sums.
Both heads are processed per t-chunk with 2-way row-strip packing (lin at
array rows 0/32, win at rows 0/64) writing the two halves of paired
[128,1024] PSUM tiles, so elementwise ops cover both heads in one
instruction.

Self-contained: only imports concourse/* from the environment.
"""

import numpy as np
import ml_dtypes

S = 2048
D = 1024
H = 16
FD = 16
HD = 64
W = 256
EPS = 1e-9
NCORES = 8

BF = ml_dtypes.bfloat16

_CACHE = {}


def _build_nc(dbg=False):
    import concourse.bass as bass
    import concourse.mybir as mybir
    import concourse.tile as tile
    from concourse import bacc
    from concourse.bass import ts

    f32 = mybir.dt.float32
    bf16 = mybir.dt.bfloat16
    MULT = mybir.AluOpType.mult
    ADD = mybir.AluOpType.add
    Exp = mybir.ActivationFunctionType.Exp
    Square = mybir.ActivationFunctionType.Square

    nc = bacc.Bacc("TRN2", target_bir_lowering=False)

    ht_d = nc.dram_tensor("ht", [D, S], bf16, kind="ExternalInput")
    wqk_d = nc.dram_tensor("wqk", [D, 384], bf16, kind="ExternalInput")
    wv_d = nc.dram_tensor("wv", [D, 256], bf16, kind="ExternalInput")
    wo_d = nc.dram_tensor("wo", [256, D], bf16, kind="ExternalInput")
    msk_d = nc.dram_tensor("msk", [128, 256], bf16, kind="ExternalInput")
    orow_d = nc.dram_tensor("orow", [1, S], bf16, kind="ExternalInput")
    out_d = nc.dram_tensor("out", [S, D], f32, kind="ExternalOutput")
    if dbg:
        dbg_t = {
            "d_qw": nc.dram_tensor("d_qw", [128, S], f32, kind="ExternalOutput"),
            "d_kw": nc.dram_tensor("d_kw", [128, S], f32, kind="ExternalOutput"),
            "d_qkg": nc.dram_tensor("d_qkg", [128, 2 * S], f32, kind="ExternalOutput"),
            "d_v4": nc.dram_tensor("d_v4", [128, 16 * 260], f32, kind="ExternalOutput"),
            "d_cum0": nc.dram_tensor("d_cum0", [65, S], f32, kind="ExternalOutput"),
            "d_ul0": nc.dram_tensor("d_ul0", [65, S], f32, kind="ExternalOutput"),
            "d_uw0": nc.dram_tensor("d_uw0", [65, S], f32, kind="ExternalOutput"),
            "d_den": nc.dram_tensor("d_den", [128, S], f32, kind="ExternalOutput"),
            "d_recf": nc.dram_tensor("d_recf", [128, S], f32, kind="ExternalOutput"),
            "d_scl0": nc.dram_tensor("d_scl0", [128, S], f32, kind="ExternalOutput"),
        }

    NJ = 4          # number of 512-wide s blocks
    SB = 512        # s block width
    NT = 16         # number of 128-wide t chunks

    with tile.TileContext(nc) as tc:
        with (
            tc.tile_pool(name="sb", bufs=1) as sb,
            tc.tile_pool(name="sqp", bufs=4) as sqp,
            tc.tile_pool(name="exp", bufs=4) as exq,
            tc.tile_pool(name="stg", bufs=3) as stg,
            tc.tile_pool(name="bct", bufs=2) as bct,
            tc.tile_pool(name="psA", bufs=2, space="PSUM") as psA,
            tc.tile_pool(name="psAcc", bufs=2, space="PSUM") as psAcc,
            tc.tile_pool(name="psO", bufs=1, space="PSUM") as psO,
        ):
            # ---------------- persistent SBUF tiles ----------------
            ht_sb = sb.tile([128, 8, S], bf16, name="ht_sb")
            wqk_sb = sb.tile([128, 8, 384], bf16, name="wqk_sb")
            wv_sb = sb.tile([128, 8, 256], bf16, name="wv_sb")
            wo_sb = sb.tile([128, 2, 1024], bf16, name="wo_sb")
            msk_sb = sb.tile([128, 256], bf16, name="msk_sb")
            # qkg: lin q/k with ones row; h0 rows 0:17, h1 rows 32:49;
            # free index 0 = q, 1 = k
            qkg_sb = sb.tile([128, 2, S], bf16, name="qkg_sb")
            qw_sb = sb.tile([128, S], bf16, name="qw_sb")
            kw_sb = sb.tile([128, S], bf16, name="kw_sb")
            v4_sb = sb.tile([128, NT, 260], bf16, name="v4_sb")
            cum_sb = [sb.tile([65, S], f32, name=f"cum{h}_sb") for h in range(2)]
            ul_sb = [sb.tile([65, S], f32, name=f"ul{h}_sb") for h in range(2)]
            uw_sb = [sb.tile([65, S], f32, name=f"uw{h}_sb") for h in range(2)]
            scl_sb = [sb.tile([128, S], bf16, name=f"scl{h}_sb") for h in range(2)]
            den_sb = sb.tile([128, S], f32, name="den_sb")
            recf_sb = sb.tile([128, S], f32, name="recf_sb")
            recr_sb = [sb.tile([1, S], f32, name=f"recr{r}_sb") for r in range(4)]

            v4r = v4_sb.rearrange("p s (g x) -> p s g x", x=65)

            # ---------------- load inputs ----------------
            for k in range(8):
                nc.sync.dma_start(out=ht_sb[:, k, :], in_=ht_d[ts(k, 128), :])
                nc.sync.dma_start(out=wqk_sb[:, k, :], in_=wqk_d[ts(k, 128), :])
                nc.sync.dma_start(out=wv_sb[:, k, :], in_=wv_d[ts(k, 128), :])
            for k in range(2):
                nc.sync.dma_start(out=wo_sb[:, k, :], in_=wo_d[ts(k, 128), :])
            nc.sync.dma_start(out=msk_sb[:, :], in_=msk_d[:, :])
            # constant columns of v4: 0.5 for lin heads, 1.0 for win heads
            nc.gpsimd.memset(v4r[:, :, 0:2, 64], 0.5)
            nc.gpsimd.memset(v4r[:, :, 2:4, 64], 1.0)
            # garbage rows of den must stay finite for the reciprocal pass
            nc.gpsimd.memset(den_sb[:, :], 1.0)

            # ---------------- phase 1a: q/k projections (paired j) ----------
            # wqk columns: [qw_a(64) qw_b(64) | kw_a(64) kw_b(64) |
            #   qlin_a@256 qlin_b@288 klin_a@320 klin_b@352 (16 each)]
            for blk in range(3):
                c0 = blk * 128
                for jp in range(2):
                    js2 = ts(jp, 1024)
                    pp = psA.tile([128, 1024], f32, name="pp", tag="psA")
                    for jh in range(2):
                        for k in range(8):
                            nc.tensor.matmul(
                                pp[:, ts(jh, SB)],
                                lhsT=wqk_sb[:, k, c0 : c0 + 128],
                                rhs=ht_sb[:, k, ts(2 * jp + jh, SB)],
                                start=(k == 0),
                                stop=(k == 7),
                            )
                    if blk == 0:
                        nc.vector.tensor_copy(out=qw_sb[:, js2], in_=pp[:, :])
                    elif blk == 1:
                        nc.vector.tensor_copy(out=kw_sb[:, js2], in_=pp[:, :])
                    else:
                        nc.vector.tensor_copy(
                            out=qkg_sb[0:48, 0, js2], in_=pp[0:48, :]
                        )
                        nc.vector.tensor_copy(
                            out=qkg_sb[0:48, 1, js2], in_=pp[64:112, :]
                        )
                        # restore ones rows clobbered by the 48-row copies
                        for qk in range(2):
                            for r in (16, 48):
                                nc.sync.dma_start(
                                    out=qkg_sb[r : r + 1, qk, js2],
                                    in_=orow_d[0:1, js2],
                                )

            # ---------------- phase 1b: v projections (paired st) -----------
            for sp_ in range(8):
                st0 = 2 * sp_
                pv = psA.tile([128, 1024], f32, name="pv", tag="psA")
                for sh in range(2):
                    for k in range(8):
                        nc.tensor.matmul(
                            pv[:, sh * 256 : sh * 256 + 256],
                            lhsT=ht_sb[:, k, ts(st0 + sh, 128)],
                            rhs=wv_sb[:, k, :],
                            start=(k == 0),
                            stop=(k == 7),
                        )
                nc.vector.tensor_copy(
                    out=v4r[:, st0 : st0 + 2, :, 0:64],
                    in_=pv[:, 0:512].rearrange("p (s g x) -> p s g x", s=2, x=64),
                )

            # ---------------- phase 1c: CUM (cumulative v-half sums) -------
            # cum[h][d, s] = sum_{t<=s} vlin_half[t, d]; row 64 = 0.5*(s+1)
            for h in range(2):
                lin_sl = slice(65 * h, 65 * h + 65)
                for sj in range(NT):
                    icp = psA.tile([128, 1024], f32, name="icp", tag="psA")
                    nc.tensor.matmul(
                        icp[0:65, 0:128],
                        lhsT=v4_sb[:, sj, lin_sl],
                        rhs=msk_sb[:, 0:128],
                        start=True,
                        stop=True,
                    )
                    if sj == 0:
                        nc.vector.tensor_scalar(
                            out=cum_sb[h][:, 0:128],
                            in0=icp[0:65, 0:128],
                            scalar1=0.0,
                            scalar2=None,
                            op0=ADD,
                        )
                    else:
                        nc.vector.tensor_scalar(
                            out=cum_sb[h][:, ts(sj, 128)],
                            in0=icp[0:65, 0:128],
                            scalar1=cum_sb[h][:, sj * 128 - 1 : sj * 128],
                            scalar2=None,
                            op0=ADD,
                        )

            # ---------------- phase 2 + pipelined tails ----------------
            # Emitted as interleaved blocks: lin/win alternate per t-chunk,
            # accumulation matmuls trail their producer by 2 blocks, and the
            # previous j-block's tail pieces are spliced into the stream so
            # no engine queues long dependency stalls.
            def phase2_blocks(j):
                js = ts(j, SB)
                lim_l = 4 * j + 4
                lim_w = min(16, 4 * j + 6)
                st_ = {}

                def begin():
                    st_["qkv"] = [
                        psAcc.tile([65, SB], f32, name=f"qkv{h}", tag="acc")
                        for h in range(2)
                    ]
                    st_["sq"] = {}

                def lin_blk(ti):
                    mp = psA.tile([128, 1024], f32, name="mp", tag="psA")
                    for h, (p0, p1) in enumerate(((0, 17), (32, 49))):
                        nc.tensor.matmul(
                            mp[:, ts(h, SB)],
                            lhsT=qkg_sb[p0:p1, 1, ts(ti, 128)],
                            rhs=qkg_sb[p0:p1, 0, js],
                            start=True,
                            stop=True,
                        )
                    sq = sqp.tile([128, 1024], bf16, name="sq", tag="sq")
                    col0 = max(0, ti - 4 * j) * 128
                    sqr = sq.rearrange("p (g x) -> p g x", x=SB)
                    mpr = mp.rearrange("p (g x) -> p g x", x=SB)
                    if col0:
                        nc.gpsimd.memset(sqr[:, :, 0:col0], 0.0)
                    if ti % 2 == 0:
                        nc.scalar.activation(
                            sqr[:, :, col0:SB], mpr[:, :, col0:SB], Square
                        )
                    else:
                        # DVE path: copy to bf16 then square in SBUF (fast mode)
                        nc.vector.tensor_copy(
                            out=sqr[:, :, col0:SB], in_=mpr[:, :, col0:SB]
                        )
                        nc.vector.tensor_tensor(
                            out=sqr[:, :, col0:SB],
                            in0=sqr[:, :, col0:SB],
                            in1=sqr[:, :, col0:SB],
                            op=MULT,
                        )
                    sd = ti - 4 * j
                    if 0 <= sd <= 3:
                        for h in range(2):
                            dsl = slice(h * SB + sd * 128, h * SB + (sd + 1) * 128)
                            nc.gpsimd.tensor_tensor(
                                out=sq[:, dsl], in0=sq[:, dsl],
                                in1=msk_sb[:, 0:128], op=MULT,
                            )
                    st_["sq"][ti] = sq

                def lin_acc(ti):
                    sqt = st_["sq"].pop(ti)
                    for h in range(2):
                        nc.tensor.matmul(
                            st_["qkv"][h][:, :],
                            lhsT=v4_sb[:, ti, slice(65 * h, 65 * h + 65)],
                            rhs=sqt[:, ts(h, SB)],
                            start=(ti == 0),
                            stop=(ti == lim_l - 1),
                            skip_group_check=True,
                        )

                def lin_end():
                    for h in range(2):
                        nc.vector.tensor_tensor(
                            out=ul_sb[h][:, js],
                            in0=st_["qkv"][h][0:65, :],
                            in1=cum_sb[h][:, js],
                            op=ADD,
                        )
                        nc.sync.dma_start(
                            out=den_sb[32 * h : 32 * h + 1, js],
                            in_=ul_sb[h][64:65, js],
                        )

                def win_begin():
                    st_["nt"] = [
                        psAcc.tile([65, SB], f32, name=f"nt{h}", tag="acc")
                        for h in range(2)
                    ]
                    st_["ex"] = {}

                def win_blk(ti):
                    sp = psA.tile([128, 1024], f32, name="sp", tag="psA")
                    for h in range(2):
                        hsl = slice(64 * h, 64 * h + 64)
                        nc.tensor.matmul(
                            sp[:, ts(h, SB)],
                            lhsT=kw_sb[hsl, ts(ti, 128)],
                            rhs=qw_sb[hsl, js],
                            start=True,
                            stop=True,
                        )
                    ex = exq.tile([128, 1024], bf16, name="ex", tag="ex")
                    col0 = max(0, ti - 2 - 4 * j) * 128
                    exr = ex.rearrange("p (g x) -> p g x", x=SB)
                    spr = sp.rearrange("p (g x) -> p g x", x=SB)
                    if col0:
                        nc.gpsimd.memset(exr[:, :, 0:col0], 0.0)
                    nc.scalar.activation(
                        exr[:, :, col0:SB], spr[:, :, col0:SB], Exp, scale=0.125
                    )
                    sd = ti - 2 - 4 * j
                    if 0 <= sd <= 3:
                        for h in range(2):
                            dsl = slice(h * SB + sd * 128, h * SB + (sd + 1) * 128)
                            nc.gpsimd.tensor_tensor(
                                out=ex[:, dsl], in0=ex[:, dsl],
                                in1=msk_sb[:, 128:256], op=MULT,
                            )
                    st_["ex"][ti] = ex

                def win_acc(ti):
                    ext = st_["ex"].pop(ti)
                    for h in range(2):
                        nc.tensor.matmul(
                            st_["nt"][h][:, :],
                            lhsT=v4_sb[:, ti, slice(130 + 65 * h, 195 + 65 * h)],
                            rhs=ext[:, ts(h, SB)],
                            start=(ti == 0),
                            stop=(ti == lim_w - 1),
                            skip_group_check=True,
                        )

                def win_end():
                    for h in range(2):
                        nc.vector.tensor_copy(
                            out=uw_sb[h][:, js], in_=st_["nt"][h][0:65, :]
                        )
                        nc.sync.dma_start(
                            out=den_sb[64 + 32 * h : 65 + 32 * h, js],
                            in_=uw_sb[h][64:65, js],
                        )

                # block list: lin chain then win chain, lag-2 accumulation
                blocks = [begin]
                for ti in range(lim_l):
                    blocks.append(lambda ti=ti: lin_blk(ti))
                    if ti >= 2:
                        blocks.append(lambda ti=ti - 2: lin_acc(ti))
                blocks.append(lambda: lin_acc(lim_l - 2))
                blocks.append(lambda: lin_acc(lim_l - 1))
                blocks.append(lin_end)
                blocks.append(win_begin)
                for ti in range(lim_w):
                    blocks.append(lambda ti=ti: win_blk(ti))
                    if ti >= 2:
                        blocks.append(lambda ti=ti - 2: win_acc(ti))
                blocks.append(lambda: win_acc(lim_w - 2))
                blocks.append(lambda: win_acc(lim_w - 1))
                blocks.append(win_end)
                return blocks

            def tail_blocks(j):
                js = ts(j, SB)
                pieces = []

                def recip():
                    nc.vector.tensor_scalar(
                        out=recf_sb[0:97, js], in0=den_sb[0:97, js],
                        scalar1=EPS, scalar2=None, op0=ADD,
                    )
                    nc.vector.reciprocal(
                        out=recf_sb[0:97, js], in_=recf_sb[0:97, js]
                    )
                    for r in range(4):
                        nc.sync.dma_start(
                            out=recr_sb[r][0:1, js],
                            in_=recf_sb[32 * r : 32 * r + 1, js],
                        )

                pieces.append(recip)

                def scale(h, p):
                    r = 2 * p + h
                    u = ul_sb[h] if p == 0 else uw_sb[h]
                    bc = bct.tile([64, SB], f32, name="bc", tag="bc")
                    nc.gpsimd.partition_broadcast(bc[:, :], recr_sb[r][0:1, js])
                    nc.gpsimd.tensor_tensor(
                        out=scl_sb[h][ts(p, 64), js],
                        in0=u[0:64, js],
                        in1=bc[:, :],
                        op=MULT,
                    )

                for h in range(2):
                    for p in range(2):
                        pieces.append(lambda h=h, p=p: scale(h, p))

                def final(st):
                    po = psO.tile([128, 1024], f32, name="po", tag="po")
                    for nb in range(2):
                        for h in range(2):
                            nc.tensor.matmul(
                                po[:, ts(nb, SB)],
                                lhsT=scl_sb[h][:, ts(st, 128)],
                                rhs=wo_sb[:, h, ts(nb, SB)],
                                start=(h == 0),
                                stop=(h == 1),
                                skip_group_check=True,
                            )
                    so = stg.tile([128, 1024], f32, name="so", tag="so")
                    nc.vector.tensor_copy(out=so[:, :], in_=po[:, :])
                    nc.sync.dma_start(out=out_d[ts(st, 128), :], in_=so[:, :])

                for st in range(4 * j, 4 * j + 4):
                    pieces.append(lambda st=st: final(st))
                return pieces

            def interleave(blocks, pieces):
                if not pieces:
                    for b in blocks:
                        b()
                    return
                stride = max(1, len(blocks) // (len(pieces) + 1))
                pi = 0
                for i, b in enumerate(blocks):
                    b()
                    if (i + 1) % stride == 0 and pi < len(pieces) and i > 2:
                        pieces[pi]()
                        pi += 1
                while pi < len(pieces):
                    pieces[pi]()
                    pi += 1

            interleave(phase2_blocks(0), [])
            interleave(phase2_blocks(1), tail_blocks(0))
            interleave(phase2_blocks(2), tail_blocks(1))
            interleave(phase2_blocks(3), tail_blocks(2))
            for p in tail_blocks(3):
                p()

    nc.compile()
    return nc


def _prep_inputs(inputs):
    """Host-side sharding/packing. Returns per-core input maps."""
    h = np.asarray(inputs["hidden_states"], np.float32).reshape(S, D)
    ht = np.ascontiguousarray(h.T).astype(BF)

    lin_Wq = np.asarray(inputs["lin_Wq"], np.float32)
    lin_Wk = np.asarray(inputs["lin_Wk"], np.float32)
    lin_Wv = np.asarray(inputs["lin_Wv"], np.float32)
    lin_Wo = np.asarray(inputs["lin_Wo"], np.float32)
    win_Wq = np.asarray(inputs["win_Wq"], np.float32)
    win_Wk = np.asarray(inputs["win_Wk"], np.float32)
    win_Wv = np.asarray(inputs["win_Wv"], np.float32)
    win_Wo = np.asarray(inputs["win_Wo"], np.float32)

    # constant mask tiles
    p = np.arange(128)[:, None]
    f = np.arange(128)[None, :]
    msk = np.zeros((128, 256), np.float32)
    msk[:, 0:128] = (p <= f)          # lin diag mask (t <= s)
    msk[:, 128:256] = (p < f)         # win partial mask (t < s)

    in_maps = []
    for c in range(NCORES):
        a, b = 2 * c, 2 * c + 1
        wqk = np.zeros((D, 384), np.float32)
        wqk[:, 0:64] = win_Wq[:, a * HD : (a + 1) * HD]
        wqk[:, 64:128] = win_Wq[:, b * HD : (b + 1) * HD]
        wqk[:, 128:192] = win_Wk[:, a * HD : (a + 1) * HD]
        wqk[:, 192:256] = win_Wk[:, b * HD : (b + 1) * HD]
        wqk[:, 256:272] = lin_Wq[:, a * FD : (a + 1) * FD] * 0.5
        wqk[:, 288:304] = lin_Wq[:, b * FD : (b + 1) * FD] * 0.5
        wqk[:, 320:336] = lin_Wk[:, a * FD : (a + 1) * FD] * 0.5
        wqk[:, 352:368] = lin_Wk[:, b * FD : (b + 1) * FD] * 0.5
        wv = np.zeros((D, 256), np.float32)
        wv[:, 0:64] = lin_Wv[:, a * HD : (a + 1) * HD] * 0.5
        wv[:, 64:128] = lin_Wv[:, b * HD : (b + 1) * HD] * 0.5
        wv[:, 128:192] = win_Wv[:, a * HD : (a + 1) * HD]
        wv[:, 192:256] = win_Wv[:, b * HD : (b + 1) * HD]
        wo = np.zeros((256, D), np.float32)
        wo[0:64] = lin_Wo[a * HD : (a + 1) * HD]
        wo[64:128] = win_Wo[a * HD : (a + 1) * HD]
        wo[128:192] = lin_Wo[b * HD : (b + 1) * HD]
        wo[192:256] = win_Wo[b * HD : (b + 1) * HD]
        in_maps.append(
            {
                "ht": ht,
                "wqk": wqk.astype(BF),
                "wv": wv.astype(BF),
                "wo": wo.astype(BF),
                "msk": msk.astype(BF),
                "orow": np.ones((1, S), np.float32).astype(BF),
            }
        )
    return in_maps


def kernel(**inputs) -> np.ndarray:
    from concourse.bass_utils import run_bass_kernel_spmd

    if "nc" not in _CACHE:
        _CACHE["nc"] = _build_nc()
    nc = _CACHE["nc"]
    in_maps = _prep_inputs(inputs)
    res = run_bass_kernel_spmd(nc, in_maps, core_ids=list(range(NCORES)))
    out = np.zeros((S, D), np.float32)
    for r in res.results:
        out += r["out"]
    return out.reshape(1, S, D)


if __name__ == "__main__":
    nc = _build_nc()
    print("built ok")


# revision 24
# speedup vs baseline: 1.0286x; 1.0286x over previous
"""Trainium2 Bass kernel for nn_Based_40630390620259 (sparse_attention).

Architecture ("Based"-style): linear (Taylor feature-map) attention +
windowed softmax attention, 16 heads, S=2048, D=1024.

Math identities used (verified against the reference to 1e-6):
  - Taylor feature map inner product collapses:
        qf.kf = 1 + (q.k)/4 + (q.k)^2/32 = 0.5 + 0.5*(1 + q.k/4)^2
    so the 273-dim feature space is never materialized. With Wq,Wk scaled
    by 0.5 on the host and a constant ones-row appended to q/k (K=17
    matmul), the PE produces m'' = 1 + q.k/4 directly; sq = m''^2 on DVE.
  - The 0.5 factor is folded into the V projection weights; the +0.5
    constant term contributes a causal cumulative sum CUM of the
    (0.5-scaled) v rows, computed with 16 N=128 matmuls against an
    upper-triangular ones block + a recursive per-partition scalar-add.
    CUM row 64 (from the 0.5-constant column) equals 0.5*(s+1), which is
    exactly the constant-term part of the reference denominator.
  - win path: scores^T computed as [t,s] tiles; softmax denominator via a
    ones-column in V'; division deferred through the output projection via
    a gpsimd partition_broadcast of the reciprocal row.

Sharding: tensor-parallel over heads, 2 heads per core, 8 cores. Each core
produces a partial [S, D] output (its heads' contribution); the host sums.
Both heads are processed per t-chunk with 2-way row-strip packing (lin at
array rows 0/32, win at rows 0/64) writing the two halves of paired
[128,1024] PSUM tiles, so elementwise ops cover both heads in one
instruction.

Self-contained: only imports concourse/* from the environment.
"""

import numpy as np
import ml_dtypes

S = 2048
D = 1024
H = 16
FD = 16
HD = 64
W = 256
EPS = 1e-9
NCORES = 8

BF = ml_dtypes.bfloat16

_CACHE = {}


def _build_nc(dbg=False):
    import concourse.bass as bass
    import concourse.mybir as mybir
    import concourse.tile as tile
    from concourse import bacc
    from concourse.bass import ts

    f32 = mybir.dt.float32
    bf16 = mybir.dt.bfloat16
    MULT = mybir.AluOpType.mult
    ADD = mybir.AluOpType.add
    Exp = mybir.ActivationFunctionType.Exp
    Square = mybir.ActivationFunctionType.Square

    nc = bacc.Bacc("TRN2", target_bir_lowering=False)

    ht_d = nc.dram_tensor("ht", [D, S], bf16, kind="ExternalInput")
    wqk_d = nc.dram_tensor("wqk", [D, 384], bf16, kind="ExternalInput")
    wv_d = nc.dram_tensor("wv", [D, 256], bf16, kind="ExternalInput")
    wo_d = nc.dram_tensor("wo", [256, D], bf16, kind="ExternalInput")
    msk_d = nc.dram_tensor("msk", [128, 256], bf16, kind="ExternalInput")
    orow_d = nc.dram_tensor("orow", [1, S], bf16, kind="ExternalInput")
    out_d = nc.dram_tensor("out", [S, D], f32, kind="ExternalOutput")
    if dbg:
        dbg_t = {
            "d_qw": nc.dram_tensor("d_qw", [128, S], f32, kind="ExternalOutput"),
            "d_kw": nc.dram_tensor("d_kw", [128, S], f32, kind="ExternalOutput"),
            "d_qkg": nc.dram_tensor("d_qkg", [128, 2 * S], f32, kind="ExternalOutput"),
            "d_v4": nc.dram_tensor("d_v4", [128, 16 * 260], f32, kind="ExternalOutput"),
            "d_cum0": nc.dram_tensor("d_cum0", [65, S], f32, kind="ExternalOutput"),
            "d_ul0": nc.dram_tensor("d_ul0", [65, S], f32, kind="ExternalOutput"),
            "d_uw0": nc.dram_tensor("d_uw0", [65, S], f32, kind="ExternalOutput"),
            "d_den": nc.dram_tensor("d_den", [128, S], f32, kind="ExternalOutput"),
            "d_recf": nc.dram_tensor("d_recf", [128, S], f32, kind="ExternalOutput"),
            "d_scl0": nc.dram_tensor("d_scl0", [128, S], f32, kind="ExternalOutput"),
        }

    NJ = 4          # number of 512-wide s blocks
    SB = 512        # s block width
    NT = 16         # number of 128-wide t chunks

    with tile.TileContext(nc) as tc:
        with (
            tc.tile_pool(name="sb", bufs=1) as sb,
            tc.tile_pool(name="sqp", bufs=5) as sqp,
            tc.tile_pool(name="exp", bufs=5) as exq,
            tc.tile_pool(name="stg", bufs=3) as stg,
            tc.tile_pool(name="bct", bufs=2) as bct,
            tc.tile_pool(name="psA", bufs=2, space="PSUM") as psA,
            tc.tile_pool(name="psAcc", bufs=2, space="PSUM") as psAcc,
            tc.tile_pool(name="psO", bufs=1, space="PSUM") as psO,
        ):
            # ---------------- persistent SBUF tiles ----------------
            ht_sb = sb.tile([128, 8, S], bf16, name="ht_sb")
            wqk_sb = sb.tile([128, 8, 384], bf16, name="wqk_sb")
            wv_sb = sb.tile([128, 8, 256], bf16, name="wv_sb")
            wo_sb = sb.tile([128, 2, 1024], bf16, name="wo_sb")
            msk_sb = sb.tile([128, 256], bf16, name="msk_sb")
            # qkg: lin q/k with ones row; h0 rows 0:17, h1 rows 32:49;
            # free index 0 = q, 1 = k
            qkg_sb = sb.tile([128, 2, S], bf16, name="qkg_sb")
            qw_sb = sb.tile([128, S], bf16, name="qw_sb")
            kw_sb = sb.tile([128, S], bf16, name="kw_sb")
            v4_sb = sb.tile([128, NT, 260], bf16, name="v4_sb")
            cum_sb = [sb.tile([65, S], f32, name=f"cum{h}_sb") for h in range(2)]
            ul_sb = [sb.tile([65, S], f32, name=f"ul{h}_sb") for h in range(2)]
            uw_sb = [sb.tile([65, S], f32, name=f"uw{h}_sb") for h in range(2)]
            scl_sb = [sb.tile([128, S], bf16, name=f"scl{h}_sb") for h in range(2)]
            den_sb = sb.tile([128, S], f32, name="den_sb")
            recf_sb = sb.tile([128, S], f32, name="recf_sb")
            recr_sb = sb.tile([1, 4, SB], f32, name="recr_sb")

            v4r = v4_sb.rearrange("p s (g x) -> p s g x", x=65)

            # ---------------- load inputs ----------------
            for k in range(8):
                nc.sync.dma_start(out=ht_sb[:, k, :], in_=ht_d[ts(k, 128), :])
                nc.sync.dma_start(out=wqk_sb[:, k, :], in_=wqk_d[ts(k, 128), :])
                nc.sync.dma_start(out=wv_sb[:, k, :], in_=wv_d[ts(k, 128), :])
            for k in range(2):
                nc.sync.dma_start(out=wo_sb[:, k, :], in_=wo_d[ts(k, 128), :])
            nc.sync.dma_start(out=msk_sb[:, :], in_=msk_d[:, :])
            # constant columns of v4: 0.5 for lin heads, 1.0 for win heads
            nc.gpsimd.memset(v4r[:, :, 0:2, 64], 0.5)
            nc.gpsimd.memset(v4r[:, :, 2:4, 64], 1.0)
            # garbage rows of den must stay finite for the reciprocal pass
            nc.gpsimd.memset(den_sb[:, :], 1.0)

            # ---------------- phase 1a: q/k projections (paired j) ----------
            # wqk columns: [qw_a(64) qw_b(64) | kw_a(64) kw_b(64) |
            #   qlin_a@256 qlin_b@288 klin_a@320 klin_b@352 (16 each)]
            for blk in range(3):
                c0 = blk * 128
                for jp in range(2):
                    js2 = ts(jp, 1024)
                    pp = psA.tile([128, 1024], f32, name="pp", tag="psA")
                    for jh in range(2):
                        for k in range(8):
                            nc.tensor.matmul(
                                pp[:, ts(jh, SB)],
                                lhsT=wqk_sb[:, k, c0 : c0 + 128],
                                rhs=ht_sb[:, k, ts(2 * jp + jh, SB)],
                                start=(k == 0),
                                stop=(k == 7),
                            )
                    if blk == 0:
                        nc.scalar.copy(out=qw_sb[:, js2], in_=pp[:, :])
                    elif blk == 1:
                        nc.scalar.copy(out=kw_sb[:, js2], in_=pp[:, :])
                    else:
                        nc.vector.tensor_copy(
                            out=qkg_sb[0:48, 0, js2], in_=pp[0:48, :]
                        )
                        nc.vector.tensor_copy(
                            out=qkg_sb[0:48, 1, js2], in_=pp[64:112, :]
                        )
                        # restore ones rows clobbered by the 48-row copies
                        for qk in range(2):
                            for r in (16, 48):
                                nc.sync.dma_start(
                                    out=qkg_sb[r : r + 1, qk, js2],
                                    in_=orow_d[0:1, js2],
                                )

            # ---------------- phase 1b: v projections (paired st) -----------
            for sp_ in range(8):
                st0 = 2 * sp_
                pv = psA.tile([128, 1024], f32, name="pv", tag="psA")
                for sh in range(2):
                    for k in range(8):
                        nc.tensor.matmul(
                            pv[:, sh * 256 : sh * 256 + 256],
                            lhsT=ht_sb[:, k, ts(st0 + sh, 128)],
                            rhs=wv_sb[:, k, :],
                            start=(k == 0),
                            stop=(k == 7),
                        )
                nc.vector.tensor_copy(
                    out=v4r[:, st0 : st0 + 2, :, 0:64],
                    in_=pv[:, 0:512].rearrange("p (s g x) -> p s g x", s=2, x=64),
                )

            # ---------------- phase 1c: CUM (cumulative v-half sums) -------
            # cum[h][d, s] = sum_{t<=s} vlin_half[t, d]; row 64 = 0.5*(s+1)
            for h in range(2):
                lin_sl = slice(65 * h, 65 * h + 65)
                for sj in range(NT):
                    icp = psA.tile([128, 1024], f32, name="icp", tag="psA")
                    nc.tensor.matmul(
                        icp[0:65, 0:128],
                        lhsT=v4_sb[:, sj, lin_sl],
                        rhs=msk_sb[:, 0:128],
                        start=True,
                        stop=True,
                    )
                    if sj == 0:
                        nc.vector.tensor_scalar(
                            out=cum_sb[h][:, 0:128],
                            in0=icp[0:65, 0:128],
                            scalar1=0.0,
                            scalar2=None,
                            op0=ADD,
                        )
                    else:
                        nc.vector.tensor_scalar(
                            out=cum_sb[h][:, ts(sj, 128)],
                            in0=icp[0:65, 0:128],
                            scalar1=cum_sb[h][:, sj * 128 - 1 : sj * 128],
                            scalar2=None,
                            op0=ADD,
                        )

            # ---------------- phase 2 + pipelined tails ----------------
            # Emitted as interleaved blocks: lin/win alternate per t-chunk,
            # accumulation matmuls trail their producer by 2 blocks, and the
            # previous j-block's tail pieces are spliced into the stream so
            # no engine queues long dependency stalls.
            def phase2_blocks(j):
                js = ts(j, SB)
                lim_l = 4 * j + 4
                lim_w = min(16, 4 * j + 6)
                st_ = {}

                def begin():
                    st_["qkv"] = [
                        psAcc.tile([65, SB], f32, name=f"qkv{h}", tag="acc")
                        for h in range(2)
                    ]
                    st_["sq"] = {}

                def lin_blk(ti):
                    mp = psA.tile([128, 1024], f32, name="mp", tag="psA")
                    for h, (p0, p1) in enumerate(((0, 17), (32, 49))):
                        nc.tensor.matmul(
                            mp[:, ts(h, SB)],
                            lhsT=qkg_sb[p0:p1, 1, ts(ti, 128)],
                            rhs=qkg_sb[p0:p1, 0, js],
                            start=True,
                            stop=True,
                        )
                    sq = sqp.tile([128, 1024], bf16, name="sq", tag="sq")
                    col0 = max(0, ti - 4 * j) * 128
                    sqr = sq.rearrange("p (g x) -> p g x", x=SB)
                    mpr = mp.rearrange("p (g x) -> p g x", x=SB)
                    if col0:
                        nc.gpsimd.memset(sqr[:, :, 0:col0], 0.0)
                    nc.scalar.activation(
                        sqr[:, :, col0:SB], mpr[:, :, col0:SB], Square
                    )
                    sd = ti - 4 * j
                    if 0 <= sd <= 3:
                        for h in range(2):
                            dsl = slice(h * SB + sd * 128, h * SB + (sd + 1) * 128)
                            nc.gpsimd.tensor_tensor(
                                out=sq[:, dsl], in0=sq[:, dsl],
                                in1=msk_sb[:, 0:128], op=MULT,
                            )
                    st_["sq"][ti] = sq

                def lin_acc(ti):
                    sqt = st_["sq"].pop(ti)
                    for h in range(2):
                        nc.tensor.matmul(
                            st_["qkv"][h][:, :],
                            lhsT=v4_sb[:, ti, slice(65 * h, 65 * h + 65)],
                            rhs=sqt[:, ts(h, SB)],
                            start=(ti == 0),
                            stop=(ti == lim_l - 1),
                            skip_group_check=True,
                        )

                def lin_end():
                    for h in range(2):
                        nc.vector.tensor_tensor(
                            out=ul_sb[h][:, js],
                            in0=st_["qkv"][h][0:65, :],
                            in1=cum_sb[h][:, js],
                            op=ADD,
                        )
                        nc.sync.dma_start(
                            out=den_sb[32 * h : 32 * h + 1, js],
                            in_=ul_sb[h][64:65, js],
                        )

                def win_begin():
                    st_["nt"] = [
                        psAcc.tile([65, SB], f32, name=f"nt{h}", tag="acc")
                        for h in range(2)
                    ]
                    st_["ex"] = {}

                def win_blk(ti):
                    sp = psA.tile([128, 1024], f32, name="sp", tag="psA")
                    for h in range(2):
                        hsl = slice(64 * h, 64 * h + 64)
                        nc.tensor.matmul(
                            sp[:, ts(h, SB)],
                            lhsT=kw_sb[hsl, ts(ti, 128)],
                            rhs=qw_sb[hsl, js],
                            start=True,
                            stop=True,
                        )
                    ex = exq.tile([128, 1024], bf16, name="ex", tag="ex")
                    col0 = max(0, ti - 2 - 4 * j) * 128
                    exr = ex.rearrange("p (g x) -> p g x", x=SB)
                    spr = sp.rearrange("p (g x) -> p g x", x=SB)
                    if col0:
                        nc.gpsimd.memset(exr[:, :, 0:col0], 0.0)
                    nc.scalar.activation(
                        exr[:, :, col0:SB], spr[:, :, col0:SB], Exp, scale=0.125
                    )
                    sd = ti - 2 - 4 * j
                    if 0 <= sd <= 3:
                        for h in range(2):
                            dsl = slice(h * SB + sd * 128, h * SB + (sd + 1) * 128)
                            nc.gpsimd.tensor_tensor(
                                out=ex[:, dsl], in0=ex[:, dsl],
                                in1=msk_sb[:, 128:256], op=MULT,
                            )
                    st_["ex"][ti] = ex

                def win_acc(ti):
                    ext = st_["ex"].pop(ti)
                    for h in range(2):
                        nc.tensor.matmul(
                            st_["nt"][h][:, :],
                            lhsT=v4_sb[:, ti, slice(130 + 65 * h, 195 + 65 * h)],
                            rhs=ext[:, ts(h, SB)],
                            start=(ti == 0),
                            stop=(ti == lim_w - 1),
                            skip_group_check=True,
                        )

                def win_end():
                    for h in range(2):
                        nc.vector.tensor_copy(
                            out=uw_sb[h][:, js], in_=st_["nt"][h][0:65, :]
                        )
                        nc.sync.dma_start(
                            out=den_sb[64 + 32 * h : 65 + 32 * h, js],
                            in_=uw_sb[h][64:65, js],
                        )

                # block list: lin chain then win chain, lag-2 accumulation
                blocks = [begin]
                for ti in range(lim_l):
                    blocks.append(lambda ti=ti: lin_blk(ti))
                    if ti >= 2:
                        blocks.append(lambda ti=ti - 2: lin_acc(ti))
                blocks.append(lambda: lin_acc(lim_l - 2))
                blocks.append(lambda: lin_acc(lim_l - 1))
                blocks.append(lin_end)
                blocks.append("WIN_START")
                blocks.append(win_begin)
                for ti in range(lim_w):
                    blocks.append(lambda ti=ti: win_blk(ti))
                    if ti >= 2:
                        blocks.append(lambda ti=ti - 2: win_acc(ti))
                blocks.append(lambda: win_acc(lim_w - 2))
                blocks.append(lambda: win_acc(lim_w - 1))
                blocks.append(win_end)
                return blocks

            def tail_blocks(j):
                js = ts(j, SB)
                pieces = []

                def recip():
                    nc.vector.tensor_scalar(
                        out=recf_sb[0:97, js], in0=den_sb[0:97, js],
                        scalar1=EPS, scalar2=None, op0=ADD,
                    )
                    nc.vector.reciprocal(
                        out=recf_sb[0:97, js], in_=recf_sb[0:97, js]
                    )
                    for r in range(4):
                        nc.sync.dma_start(
                            out=recr_sb[0:1, r, :],
                            in_=recf_sb[32 * r : 32 * r + 1, js],
                        )

                pieces.append(recip)

                def scale(h, p):
                    r = 2 * p + h
                    u = ul_sb[h] if p == 0 else uw_sb[h]
                    bc = bct.tile([64, SB], f32, name="bc", tag="bc")
                    nc.gpsimd.partition_broadcast(bc[:, :], recr_sb[0:1, r, :])
                    nc.gpsimd.tensor_tensor(
                        out=scl_sb[h][ts(p, 64), js],
                        in0=u[0:64, js],
                        in1=bc[:, :],
                        op=MULT,
                    )

                for h in range(2):
                    for p in range(2):
                        pieces.append(lambda h=h, p=p: scale(h, p))

                def final(st):
                    po = psO.tile([128, 1024], f32, name="po", tag="po")
                    for nb in range(2):
                        for h in range(2):
                            nc.tensor.matmul(
                                po[:, ts(nb, SB)],
                                lhsT=scl_sb[h][:, ts(st, 128)],
                                rhs=wo_sb[:, h, ts(nb, SB)],
                                start=(h == 0),
                                stop=(h == 1),
                                skip_group_check=True,
                            )
                    so = stg.tile([128, 1024], f32, name="so", tag="so")
                    nc.vector.tensor_copy(out=so[:, :], in_=po[:, :])
                    nc.sync.dma_start(out=out_d[ts(st, 128), :], in_=so[:, :])

                for st in range(4 * j, 4 * j + 4):
                    pieces.append(lambda st=st: final(st))
                return pieces

            def interleave(blocks, pieces):
                wstart = blocks.index("WIN_START") if "WIN_START" in blocks else 0
                blocks = [b for b in blocks if b != "WIN_START"]
                if not pieces:
                    for b in blocks:
                        b()
                    return
                nwin = len(blocks) - wstart
                stride = max(1, nwin // (len(pieces) + 1))
                pi = 0
                for i, b in enumerate(blocks):
                    b()
                    if i >= wstart and (i - wstart + 1) % stride == 0 and pi < len(pieces):
                        pieces[pi]()
                        pi += 1
                while pi < len(pieces):
                    pieces[pi]()
                    pi += 1

            interleave(phase2_blocks(0), [])
            interleave(phase2_blocks(1), tail_blocks(0))
            interleave(phase2_blocks(2), tail_blocks(1))
            interleave(phase2_blocks(3), tail_blocks(2))
            for p in tail_blocks(3):
                p()

    nc.compile()
    return nc


def _prep_inputs(inputs):
    """Host-side sharding/packing. Returns per-core input maps."""
    h = np.asarray(inputs["hidden_states"], np.float32).reshape(S, D)
    ht = np.ascontiguousarray(h.T).astype(BF)

    lin_Wq = np.asarray(inputs["lin_Wq"], np.float32)
    lin_Wk = np.asarray(inputs["lin_Wk"], np.float32)
    lin_Wv = np.asarray(inputs["lin_Wv"], np.float32)
    lin_Wo = np.asarray(inputs["lin_Wo"], np.float32)
    win_Wq = np.asarray(inputs["win_Wq"], np.float32)
    win_Wk = np.asarray(inputs["win_Wk"], np.float32)
    win_Wv = np.asarray(inputs["win_Wv"], np.float32)
    win_Wo = np.asarray(inputs["win_Wo"], np.float32)

    # constant mask tiles
    p = np.arange(128)[:, None]
    f = np.arange(128)[None, :]
    msk = np.zeros((128, 256), np.float32)
    msk[:, 0:128] = (p <= f)          # lin diag mask (t <= s)
    msk[:, 128:256] = (p < f)         # win partial mask (t < s)

    in_maps = []
    for c in range(NCORES):
        a, b = 2 * c, 2 * c + 1
        wqk = np.zeros((D, 384), np.float32)
        wqk[:, 0:64] = win_Wq[:, a * HD : (a + 1) * HD]
        wqk[:, 64:128] = win_Wq[:, b * HD : (b + 1) * HD]
        wqk[:, 128:192] = win_Wk[:, a * HD : (a + 1) * HD]
        wqk[:, 192:256] = win_Wk[:, b * HD : (b + 1) * HD]
        wqk[:, 256:272] = lin_Wq[:, a * FD : (a + 1) * FD] * 0.5
        wqk[:, 288:304] = lin_Wq[:, b * FD : (b + 1) * FD] * 0.5
        wqk[:, 320:336] = lin_Wk[:, a * FD : (a + 1) * FD] * 0.5
        wqk[:, 352:368] = lin_Wk[:, b * FD : (b + 1) * FD] * 0.5
        wv = np.zeros((D, 256), np.float32)
        wv[:, 0:64] = lin_Wv[:, a * HD : (a + 1) * HD] * 0.5
        wv[:, 64:128] = lin_Wv[:, b * HD : (b + 1) * HD] * 0.5
        wv[:, 128:192] = win_Wv[:, a * HD : (a + 1) * HD]
        wv[:, 192:256] = win_Wv[:, b * HD : (b + 1) * HD]
        wo = np.zeros((256, D), np.float32)
        wo[0:64] = lin_Wo[a * HD : (a + 1) * HD]
        wo[64:128] = win_Wo[a * HD : (a + 1) * HD]
        wo[128:192] = lin_Wo[b * HD : (b + 1) * HD]
        wo[192:256] = win_Wo[b * HD : (b + 1) * HD]
        in_maps.append(
            {
                "ht": ht,
                "wqk": wqk.astype(BF),
                "wv": wv.astype(BF),
                "wo": wo.astype(BF),
                "msk": msk.astype(BF),
                "orow": np.ones((1, S), np.float32).astype(BF),
            }
        )
    return in_maps


def kernel(**inputs) -> np.ndarray:
    from concourse.bass_utils import run_bass_kernel_spmd

    if "nc" not in _CACHE:
        _CACHE["nc"] = _build_nc()
    nc = _CACHE["nc"]
    in_maps = _prep_inputs(inputs)
    res = run_bass_kernel_spmd(nc, in_maps, core_ids=list(range(NCORES)))
    out = np.zeros((S, D), np.float32)
    for r in res.results:
        out += r["out"]
    return out.reshape(1, S, D)


if __name__ == "__main__":
    nc = _build_nc()
    print("built ok")


# revision 26
# speedup vs baseline: 1.0824x; 1.0523x over previous
"""Trainium2 Bass kernel for nn_Based_40630390620259 (sparse_attention).

Architecture ("Based"-style): linear (Taylor feature-map) attention +
windowed softmax attention, 16 heads, S=2048, D=1024.

Math identities used (verified against the reference to 1e-6):
  - Taylor feature map inner product collapses:
        qf.kf = 1 + (q.k)/4 + (q.k)^2/32 = 0.5 + 0.5*(1 + q.k/4)^2
    so the 273-dim feature space is never materialized. With Wq,Wk scaled
    by 0.5 on the host and a constant ones-row appended to q/k (K=17
    matmul), the PE produces m'' = 1 + q.k/4 directly; sq = m''^2 on DVE.
  - The 0.5 factor is folded into the V projection weights; the +0.5
    constant term contributes a causal cumulative sum CUM of the
    (0.5-scaled) v rows, computed with 16 N=128 matmuls against an
    upper-triangular ones block + a recursive per-partition scalar-add.
    CUM row 64 (from the 0.5-constant column) equals 0.5*(s+1), which is
    exactly the constant-term part of the reference denominator.
  - win path: scores^T computed as [t,s] tiles; softmax denominator via a
    ones-column in V'; division deferred through the output projection via
    a gpsimd partition_broadcast of the reciprocal row.

Sharding: tensor-parallel over heads, 2 heads per core, 8 cores. Each core
produces a partial [S, D] output (its heads' contribution); the host sums.
Both heads are processed per t-chunk with 2-way row-strip packing (lin at
array rows 0/32, win at rows 0/64) writing the two halves of paired
[128,1024] PSUM tiles, so elementwise ops cover both heads in one
instruction.

Self-contained: only imports concourse/* from the environment.
"""

import numpy as np
import ml_dtypes

S = 2048
D = 1024
H = 16
FD = 16
HD = 64
W = 256
EPS = 1e-9
NCORES = 8

BF = ml_dtypes.bfloat16

_CACHE = {}


def _build_nc(dbg=False):
    import concourse.bass as bass
    import concourse.mybir as mybir
    import concourse.tile as tile
    from concourse import bacc
    from concourse.bass import ts

    f32 = mybir.dt.float32
    bf16 = mybir.dt.bfloat16
    MULT = mybir.AluOpType.mult
    ADD = mybir.AluOpType.add
    Exp = mybir.ActivationFunctionType.Exp
    Square = mybir.ActivationFunctionType.Square

    nc = bacc.Bacc("TRN2", target_bir_lowering=False)

    ht_d = nc.dram_tensor("ht", [D, S], bf16, kind="ExternalInput")
    wqk_d = nc.dram_tensor("wqk", [D, 384], bf16, kind="ExternalInput")
    wv_d = nc.dram_tensor("wv", [D, 256], bf16, kind="ExternalInput")
    wo_d = nc.dram_tensor("wo", [256, D], bf16, kind="ExternalInput")
    msk_d = nc.dram_tensor("msk", [128, 256], bf16, kind="ExternalInput")
    orow_d = nc.dram_tensor("orow", [1, S], bf16, kind="ExternalInput")
    out_d = nc.dram_tensor("out", [S, D], f32, kind="ExternalOutput")
    if dbg:
        dbg_t = {
            "d_qw": nc.dram_tensor("d_qw", [128, S], f32, kind="ExternalOutput"),
            "d_kw": nc.dram_tensor("d_kw", [128, S], f32, kind="ExternalOutput"),
            "d_qkg": nc.dram_tensor("d_qkg", [128, 2 * S], f32, kind="ExternalOutput"),
            "d_v4": nc.dram_tensor("d_v4", [128, 16 * 260], f32, kind="ExternalOutput"),
            "d_cum0": nc.dram_tensor("d_cum0", [65, S], f32, kind="ExternalOutput"),
            "d_ul0": nc.dram_tensor("d_ul0", [65, S], f32, kind="ExternalOutput"),
            "d_uw0": nc.dram_tensor("d_uw0", [65, S], f32, kind="ExternalOutput"),
            "d_den": nc.dram_tensor("d_den", [128, S], f32, kind="ExternalOutput"),
            "d_recf": nc.dram_tensor("d_recf", [128, S], f32, kind="ExternalOutput"),
            "d_scl0": nc.dram_tensor("d_scl0", [128, S], f32, kind="ExternalOutput"),
        }

    NJ = 4          # number of 512-wide s blocks
    SB = 512        # s block width
    NT = 16         # number of 128-wide t chunks

    with tile.TileContext(nc) as tc:
        with (
            tc.tile_pool(name="sb", bufs=1) as sb,
            tc.tile_pool(name="sqp", bufs=5) as sqp,
            tc.tile_pool(name="exp", bufs=5) as exq,
            tc.tile_pool(name="stg", bufs=3) as stg,
            tc.tile_pool(name="bct", bufs=2) as bct,
            tc.tile_pool(name="psA", bufs=2, space="PSUM") as psA,
            tc.tile_pool(name="psAcc", bufs=2, space="PSUM") as psAcc,
            tc.tile_pool(name="psO", bufs=1, space="PSUM") as psO,
        ):
            # ---------------- persistent SBUF tiles ----------------
            ht_sb = sb.tile([128, 8, S], bf16, name="ht_sb")
            wqk_sb = sb.tile([128, 8, 384], bf16, name="wqk_sb")
            wv_sb = sb.tile([128, 8, 256], bf16, name="wv_sb")
            wo_sb = sb.tile([128, 2, 1024], bf16, name="wo_sb")
            msk_sb = sb.tile([128, 256], bf16, name="msk_sb")
            # qkg: lin q/k with ones row; h0 rows 0:17, h1 rows 32:49;
            # free index 0 = q, 1 = k
            qkg_sb = sb.tile([128, 2, S], bf16, name="qkg_sb")
            qw_sb = sb.tile([128, S], bf16, name="qw_sb")
            kw_sb = sb.tile([128, S], bf16, name="kw_sb")
            v4_sb = sb.tile([128, NT, 260], bf16, name="v4_sb")
            cum_sb = [sb.tile([65, S], f32, name=f"cum{h}_sb") for h in range(2)]
            ul_sb = [sb.tile([65, S], f32, name=f"ul{h}_sb") for h in range(2)]
            uw_sb = [sb.tile([65, S], f32, name=f"uw{h}_sb") for h in range(2)]
            scl_sb = [sb.tile([128, S], bf16, name=f"scl{h}_sb") for h in range(2)]
            den_sb = sb.tile([128, S], f32, name="den_sb")
            recf_sb = sb.tile([128, S], f32, name="recf_sb")
            recr_sb = sb.tile([1, 4, SB], f32, name="recr_sb")

            v4r = v4_sb.rearrange("p s (g x) -> p s g x", x=65)

            # ---------------- load inputs ----------------
            for k in range(8):
                nc.sync.dma_start(
                    out=ht_sb[:, k, 0:1024], in_=ht_d[ts(k, 128), 0:1024]
                )
                nc.sync.dma_start(out=wqk_sb[:, k, :], in_=wqk_d[ts(k, 128), :])
                nc.sync.dma_start(out=wv_sb[:, k, :], in_=wv_d[ts(k, 128), :])
            for k in range(8):
                nc.sync.dma_start(
                    out=ht_sb[:, k, 1024:2048], in_=ht_d[ts(k, 128), 1024:2048]
                )
            for k in range(2):
                nc.sync.dma_start(out=wo_sb[:, k, :], in_=wo_d[ts(k, 128), :])
            nc.sync.dma_start(out=msk_sb[:, :], in_=msk_d[:, :])
            # constant columns of v4: 0.5 for lin heads, 1.0 for win heads
            nc.gpsimd.memset(v4r[:, :, 0:2, 64], 0.5)
            nc.gpsimd.memset(v4r[:, :, 2:4, 64], 1.0)
            # garbage rows of den/recf must stay finite and initialized
            nc.gpsimd.memset(den_sb[:, :], 1.0)
            nc.gpsimd.memset(recf_sb[:, :], 1.0)

            # ---------------- phase 1a: q/k projections (paired j) ----------
            # wqk columns: [qw_a(64) qw_b(64) | kw_a(64) kw_b(64) |
            #   qlin_a@256 qlin_b@288 klin_a@320 klin_b@352 (16 each)]
            for blk in range(3):
                c0 = blk * 128
                for jp in range(2):
                    js2 = ts(jp, 1024)
                    pp = psA.tile([128, 1024], f32, name="pp", tag="psA")
                    for jh in range(2):
                        for k in range(8):
                            nc.tensor.matmul(
                                pp[:, ts(jh, SB)],
                                lhsT=wqk_sb[:, k, c0 : c0 + 128],
                                rhs=ht_sb[:, k, ts(2 * jp + jh, SB)],
                                start=(k == 0),
                                stop=(k == 7),
                            )
                    if blk == 0:
                        nc.scalar.copy(out=qw_sb[:, js2], in_=pp[:, :])
                    elif blk == 1:
                        nc.scalar.copy(out=kw_sb[:, js2], in_=pp[:, :])
                    else:
                        nc.vector.tensor_copy(
                            out=qkg_sb[0:48, 0, js2], in_=pp[0:48, :]
                        )
                        nc.vector.tensor_copy(
                            out=qkg_sb[0:48, 1, js2], in_=pp[64:112, :]
                        )
                        # restore ones rows clobbered by the 48-row copies
                        for qk in range(2):
                            for r in (16, 48):
                                nc.sync.dma_start(
                                    out=qkg_sb[r : r + 1, qk, js2],
                                    in_=orow_d[0:1, js2],
                                )

            # ---------------- phase 1b: v projections (paired st) -----------
            for sp_ in range(8):
                st0 = 2 * sp_
                pv = psA.tile([128, 1024], f32, name="pv", tag="psA")
                for sh in range(2):
                    for k in range(8):
                        nc.tensor.matmul(
                            pv[:, sh * 256 : sh * 256 + 256],
                            lhsT=ht_sb[:, k, ts(st0 + sh, 128)],
                            rhs=wv_sb[:, k, :],
                            start=(k == 0),
                            stop=(k == 7),
                        )
                nc.vector.tensor_copy(
                    out=v4r[:, st0 : st0 + 2, :, 0:64],
                    in_=pv[:, 0:512].rearrange("p (s g x) -> p s g x", s=2, x=64),
                )

            # ---------------- phase 1c: CUM (cumulative v-half sums) -------
            # cum[h][d, s] = sum_{t<=s} vlin_half[t, d]; row 64 = 0.5*(s+1)
            for h in range(2):
                lin_sl = slice(65 * h, 65 * h + 65)
                for sj in range(NT):
                    icp = psA.tile([128, 1024], f32, name="icp", tag="psA")
                    nc.tensor.matmul(
                        icp[0:65, 0:128],
                        lhsT=v4_sb[:, sj, lin_sl],
                        rhs=msk_sb[:, 0:128],
                        start=True,
                        stop=True,
                    )
                    if sj == 0:
                        nc.vector.tensor_scalar(
                            out=cum_sb[h][:, 0:128],
                            in0=icp[0:65, 0:128],
                            scalar1=0.0,
                            scalar2=None,
                            op0=ADD,
                        )
                    else:
                        nc.vector.tensor_scalar(
                            out=cum_sb[h][:, ts(sj, 128)],
                            in0=icp[0:65, 0:128],
                            scalar1=cum_sb[h][:, sj * 128 - 1 : sj * 128],
                            scalar2=None,
                            op0=ADD,
                        )

            # ---------------- phase 2 + pipelined tails ----------------
            # Emitted as interleaved blocks: lin/win alternate per t-chunk,
            # accumulation matmuls trail their producer by 2 blocks, and the
            # previous j-block's tail pieces are spliced into the stream so
            # no engine queues long dependency stalls.
            def phase2_blocks(j):
                js = ts(j, SB)
                lim_l = 4 * j + 4
                lim_w = min(16, 4 * j + 6)
                st_ = {}

                def begin():
                    st_["qkv"] = [
                        psAcc.tile([65, SB], f32, name=f"qkv{h}", tag="acc")
                        for h in range(2)
                    ]
                    st_["sq"] = {}

                def lin_blk(ti):
                    mp = psA.tile([128, 1024], f32, name="mp", tag="psA")
                    for h, (p0, p1) in enumerate(((0, 17), (32, 49))):
                        nc.tensor.matmul(
                            mp[:, ts(h, SB)],
                            lhsT=qkg_sb[p0:p1, 1, ts(ti, 128)],
                            rhs=qkg_sb[p0:p1, 0, js],
                            start=True,
                            stop=True,
                        )
                    sq = sqp.tile([128, 1024], bf16, name="sq", tag="sq")
                    col0 = max(0, ti - 4 * j) * 128
                    sqr = sq.rearrange("p (g x) -> p g x", x=SB)
                    mpr = mp.rearrange("p (g x) -> p g x", x=SB)
                    if col0:
                        nc.gpsimd.memset(sqr[:, :, 0:col0], 0.0)
                    nc.scalar.activation(
                        sqr[:, :, col0:SB], mpr[:, :, col0:SB], Square
                    )
                    sd = ti - 4 * j
                    if 0 <= sd <= 3:
                        for h in range(2):
                            dsl = slice(h * SB + sd * 128, h * SB + (sd + 1) * 128)
                            nc.gpsimd.tensor_tensor(
                                out=sq[:, dsl], in0=sq[:, dsl],
                                in1=msk_sb[:, 0:128], op=MULT,
                            )
                    st_["sq"][ti] = sq

                def lin_acc(ti):
                    sqt = st_["sq"].pop(ti)
                    for h in range(2):
                        nc.tensor.matmul(
                            st_["qkv"][h][:, :],
                            lhsT=v4_sb[:, ti, slice(65 * h, 65 * h + 65)],
                            rhs=sqt[:, ts(h, SB)],
                            start=(ti == 0),
                            stop=(ti == lim_l - 1),
                            skip_group_check=True,
                        )

                def lin_end():
                    for h in range(2):
                        nc.vector.tensor_tensor(
                            out=ul_sb[h][:, js],
                            in0=st_["qkv"][h][0:65, :],
                            in1=cum_sb[h][:, js],
                            op=ADD,
                        )
                        nc.sync.dma_start(
                            out=den_sb[32 * h : 32 * h + 1, js],
                            in_=ul_sb[h][64:65, js],
                        )

                def win_begin():
                    st_["nt"] = [
                        psAcc.tile([65, SB], f32, name=f"nt{h}", tag="acc")
                        for h in range(2)
                    ]
                    st_["ex"] = {}

                def win_blk(ti):
                    sp = psA.tile([128, 1024], f32, name="sp", tag="psA")
                    for h in range(2):
                        hsl = slice(64 * h, 64 * h + 64)
                        nc.tensor.matmul(
                            sp[:, ts(h, SB)],
                            lhsT=kw_sb[hsl, ts(ti, 128)],
                            rhs=qw_sb[hsl, js],
                            start=True,
                            stop=True,
                        )
                    ex = exq.tile([128, 1024], bf16, name="ex", tag="ex")
                    col0 = max(0, ti - 2 - 4 * j) * 128
                    exr = ex.rearrange("p (g x) -> p g x", x=SB)
                    spr = sp.rearrange("p (g x) -> p g x", x=SB)
                    if col0:
                        nc.gpsimd.memset(exr[:, :, 0:col0], 0.0)
                    nc.scalar.activation(
                        exr[:, :, col0:SB], spr[:, :, col0:SB], Exp, scale=0.125
                    )
                    sd = ti - 2 - 4 * j
                    if 0 <= sd <= 3:
                        for h in range(2):
                            dsl = slice(h * SB + sd * 128, h * SB + (sd + 1) * 128)
                            nc.gpsimd.tensor_tensor(
                                out=ex[:, dsl], in0=ex[:, dsl],
                                in1=msk_sb[:, 128:256], op=MULT,
                            )
                    st_["ex"][ti] = ex

                def win_acc(ti):
                    ext = st_["ex"].pop(ti)
                    for h in range(2):
                        nc.tensor.matmul(
                            st_["nt"][h][:, :],
                            lhsT=v4_sb[:, ti, slice(130 + 65 * h, 195 + 65 * h)],
                            rhs=ext[:, ts(h, SB)],
                            start=(ti == 0),
                            stop=(ti == lim_w - 1),
                            skip_group_check=True,
                        )

                def win_end():
                    for h in range(2):
                        nc.vector.tensor_copy(
                            out=uw_sb[h][:, js], in_=st_["nt"][h][0:65, :]
                        )
                        nc.sync.dma_start(
                            out=den_sb[64 + 32 * h : 65 + 32 * h, js],
                            in_=uw_sb[h][64:65, js],
                        )

                # block list: lin chain then win chain, lag-2 accumulation
                blocks = [begin]
                for ti in range(lim_l):
                    blocks.append(lambda ti=ti: lin_blk(ti))
                    if ti >= 2:
                        blocks.append(lambda ti=ti - 2: lin_acc(ti))
                blocks.append(lambda: lin_acc(lim_l - 2))
                blocks.append(lambda: lin_acc(lim_l - 1))
                blocks.append(lin_end)
                blocks.append("WIN_START")
                blocks.append(win_begin)
                for ti in range(lim_w):
                    blocks.append(lambda ti=ti: win_blk(ti))
                    if ti >= 2:
                        blocks.append(lambda ti=ti - 2: win_acc(ti))
                blocks.append(lambda: win_acc(lim_w - 2))
                blocks.append(lambda: win_acc(lim_w - 1))
                blocks.append(win_end)
                return blocks

            recf4v = None

            def tail_lin_pieces(j):
                js = ts(j, SB)
                pieces = []

                def lin_recip():
                    nc.vector.tensor_scalar(
                        out=recf_sb[0:33, js], in0=den_sb[0:33, js],
                        scalar1=EPS, scalar2=None, op0=ADD,
                    )
                    nc.vector.reciprocal(
                        out=recf_sb[0:33, js], in_=recf_sb[0:33, js]
                    )
                    rv = recf_sb.rearrange("(a b) f -> a b f", b=32)
                    nc.sync.dma_start(
                        out=recr_sb[0:1, 0:2, :], in_=rv[0:2, 0, js]
                    )

                pieces.append(lin_recip)

                def scale(h):
                    bc = bct.tile([64, SB], f32, name="bc", tag="bc")
                    nc.gpsimd.partition_broadcast(bc[:, :], recr_sb[0:1, h, :])
                    nc.vector.tensor_tensor(
                        out=scl_sb[h][0:64, js],
                        in0=ul_sb[h][0:64, js],
                        in1=bc[:, :],
                        op=MULT,
                    )

                for h in range(2):
                    pieces.append(lambda h=h: scale(h))
                return pieces

            def tail_win_pieces(j):
                js = ts(j, SB)
                pieces = []

                def win_recip():
                    nc.vector.tensor_scalar(
                        out=recf_sb[64:97, js], in0=den_sb[64:97, js],
                        scalar1=EPS, scalar2=None, op0=ADD,
                    )
                    nc.vector.reciprocal(
                        out=recf_sb[64:97, js], in_=recf_sb[64:97, js]
                    )
                    rv = recf_sb.rearrange("(a b) f -> a b f", b=32)
                    nc.sync.dma_start(
                        out=recr_sb[0:1, 2:4, :], in_=rv[2:4, 0, js]
                    )

                pieces.append(win_recip)

                def scale(h):
                    bc = bct.tile([64, SB], f32, name="bc", tag="bc")
                    nc.gpsimd.partition_broadcast(bc[:, :], recr_sb[0:1, 2 + h, :])
                    nc.vector.tensor_tensor(
                        out=scl_sb[h][64:128, js],
                        in0=uw_sb[h][0:64, js],
                        in1=bc[:, :],
                        op=MULT,
                    )

                for h in range(2):
                    pieces.append(lambda h=h: scale(h))

                def final(st):
                    po = psO.tile([128, 1024], f32, name="po", tag="po")
                    for nb in range(2):
                        for h in range(2):
                            nc.tensor.matmul(
                                po[:, ts(nb, SB)],
                                lhsT=scl_sb[h][:, ts(st, 128)],
                                rhs=wo_sb[:, h, ts(nb, SB)],
                                start=(h == 0),
                                stop=(h == 1),
                                skip_group_check=True,
                            )
                    so = stg.tile([128, 1024], f32, name="so", tag="so")
                    nc.vector.tensor_copy(out=so[:, :], in_=po[:, :])
                    nc.sync.dma_start(out=out_d[ts(st, 128), :], in_=so[:, :])

                for st in range(4 * j, 4 * j + 4):
                    pieces.append(lambda st=st: final(st))
                return pieces

            def interleave(blocks, pieces):
                wstart = blocks.index("WIN_START") if "WIN_START" in blocks else 0
                blocks = [b for b in blocks if b != "WIN_START"]
                if not pieces:
                    for b in blocks:
                        b()
                    return
                nwin = len(blocks) - wstart
                stride = max(1, nwin // (len(pieces) + 1))
                pi = 0
                for i, b in enumerate(blocks):
                    b()
                    if i >= wstart and (i - wstart + 1) % stride == 0 and pi < len(pieces):
                        pieces[pi]()
                        pi += 1
                while pi < len(pieces):
                    pieces[pi]()
                    pi += 1

            interleave(phase2_blocks(0), tail_lin_pieces(0))
            interleave(phase2_blocks(1), tail_win_pieces(0) + tail_lin_pieces(1))
            interleave(phase2_blocks(2), tail_win_pieces(1) + tail_lin_pieces(2))
            interleave(phase2_blocks(3), tail_win_pieces(2) + tail_lin_pieces(3))
            for p in tail_win_pieces(3):
                p()

    nc.compile()
    return nc


def _prep_inputs(inputs):
    """Host-side sharding/packing. Returns per-core input maps."""
    h = np.asarray(inputs["hidden_states"], np.float32).reshape(S, D)
    ht = np.ascontiguousarray(h.T).astype(BF)

    lin_Wq = np.asarray(inputs["lin_Wq"], np.float32)
    lin_Wk = np.asarray(inputs["lin_Wk"], np.float32)
    lin_Wv = np.asarray(inputs["lin_Wv"], np.float32)
    lin_Wo = np.asarray(inputs["lin_Wo"], np.float32)
    win_Wq = np.asarray(inputs["win_Wq"], np.float32)
    win_Wk = np.asarray(inputs["win_Wk"], np.float32)
    win_Wv = np.asarray(inputs["win_Wv"], np.float32)
    win_Wo = np.asarray(inputs["win_Wo"], np.float32)

    # constant mask tiles
    p = np.arange(128)[:, None]
    f = np.arange(128)[None, :]
    msk = np.zeros((128, 256), np.float32)
    msk[:, 0:128] = (p <= f)          # lin diag mask (t <= s)
    msk[:, 128:256] = (p < f)         # win partial mask (t < s)

    in_maps = []
    for c in range(NCORES):
        a, b = 2 * c, 2 * c + 1
        wqk = np.zeros((D, 384), np.float32)
        wqk[:, 0:64] = win_Wq[:, a * HD : (a + 1) * HD]
        wqk[:, 64:128] = win_Wq[:, b * HD : (b + 1) * HD]
        wqk[:, 128:192] = win_Wk[:, a * HD : (a + 1) * HD]
        wqk[:, 192:256] = win_Wk[:, b * HD : (b + 1) * HD]
        wqk[:, 256:272] = lin_Wq[:, a * FD : (a + 1) * FD] * 0.5
        wqk[:, 288:304] = lin_Wq[:, b * FD : (b + 1) * FD] * 0.5
        wqk[:, 320:336] = lin_Wk[:, a * FD : (a + 1) * FD] * 0.5
        wqk[:, 352:368] = lin_Wk[:, b * FD : (b + 1) * FD] * 0.5
        wv = np.zeros((D, 256), np.float32)
        wv[:, 0:64] = lin_Wv[:, a * HD : (a + 1) * HD] * 0.5
        wv[:, 64:128] = lin_Wv[:, b * HD : (b + 1) * HD] * 0.5
        wv[:, 128:192] = win_Wv[:, a * HD : (a + 1) * HD]
        wv[:, 192:256] = win_Wv[:, b * HD : (b + 1) * HD]
        wo = np.zeros((256, D), np.float32)
        wo[0:64] = lin_Wo[a * HD : (a + 1) * HD]
        wo[64:128] = win_Wo[a * HD : (a + 1) * HD]
        wo[128:192] = lin_Wo[b * HD : (b + 1) * HD]
        wo[192:256] = win_Wo[b * HD : (b + 1) * HD]
        in_maps.append(
            {
                "ht": ht,
                "wqk": wqk.astype(BF),
                "wv": wv.astype(BF),
                "wo": wo.astype(BF),
                "msk": msk.astype(BF),
                "orow": np.ones((1, S), np.float32).astype(BF),
            }
        )
    return in_maps


def kernel(**inputs) -> np.ndarray:
    from concourse.bass_utils import run_bass_kernel_spmd

    if "nc" not in _CACHE:
        _CACHE["nc"] = _build_nc()
    nc = _CACHE["nc"]
    in_maps = _prep_inputs(inputs)
    res = run_bass_kernel_spmd(nc, in_maps, core_ids=list(range(NCORES)))
    out = np.zeros((S, D), np.float32)
    for r in res.results:
        out += r["out"]
    return out.reshape(1, S, D)


if __name__ == "__main__":
    nc = _build_nc()
    print("built ok")


# revision 27
# speedup vs baseline: 1.0934x; 1.0102x over previous
"""Trainium2 Bass kernel for nn_Based_40630390620259 (sparse_attention).

Architecture ("Based"-style): linear (Taylor feature-map) attention +
windowed softmax attention, 16 heads, S=2048, D=1024.

Math identities used (verified against the reference to 1e-6):
  - Taylor feature map inner product collapses:
        qf.kf = 1 + (q.k)/4 + (q.k)^2/32 = 0.5 + 0.5*(1 + q.k/4)^2
    so the 273-dim feature space is never materialized. With Wq,Wk scaled
    by 0.5 on the host and a constant ones-row appended to q/k (K=17
    matmul), the PE produces m'' = 1 + q.k/4 directly; sq = m''^2 on DVE.
  - The 0.5 factor is folded into the V projection weights; the +0.5
    constant term contributes a causal cumulative sum CUM of the
    (0.5-scaled) v rows, computed with 16 N=128 matmuls against an
    upper-triangular ones block + a recursive per-partition scalar-add.
    CUM row 64 (from the 0.5-constant column) equals 0.5*(s+1), which is
    exactly the constant-term part of the reference denominator.
  - win path: scores^T computed as [t,s] tiles; softmax denominator via a
    ones-column in V'; division deferred through the output projection via
    a gpsimd partition_broadcast of the reciprocal row.

Sharding: tensor-parallel over heads, 2 heads per core, 8 cores. Each core
produces a partial [S, D] output (its heads' contribution); the host sums.
Both heads are processed per t-chunk with 2-way row-strip packing (lin at
array rows 0/32, win at rows 0/64) writing the two halves of paired
[128,1024] PSUM tiles, so elementwise ops cover both heads in one
instruction.

Self-contained: only imports concourse/* from the environment.
"""

import numpy as np
import ml_dtypes

S = 2048
D = 1024
H = 16
FD = 16
HD = 64
W = 256
EPS = 1e-9
NCORES = 8

BF = ml_dtypes.bfloat16

_CACHE = {}


def _build_nc(dbg=False):
    import concourse.bass as bass
    import concourse.mybir as mybir
    import concourse.tile as tile
    from concourse import bacc
    from concourse.bass import ts

    f32 = mybir.dt.float32
    bf16 = mybir.dt.bfloat16
    MULT = mybir.AluOpType.mult
    ADD = mybir.AluOpType.add
    Exp = mybir.ActivationFunctionType.Exp
    Square = mybir.ActivationFunctionType.Square

    nc = bacc.Bacc("TRN2", target_bir_lowering=False)

    ht_d = nc.dram_tensor("ht", [D, S], bf16, kind="ExternalInput")
    wqk_d = nc.dram_tensor("wqk", [D, 384], bf16, kind="ExternalInput")
    wv_d = nc.dram_tensor("wv", [D, 256], bf16, kind="ExternalInput")
    wo_d = nc.dram_tensor("wo", [256, D], bf16, kind="ExternalInput")
    msk_d = nc.dram_tensor("msk", [128, 256], bf16, kind="ExternalInput")
    orow_d = nc.dram_tensor("orow", [1, S], bf16, kind="ExternalInput")
    out_d = nc.dram_tensor("out", [S, D], f32, kind="ExternalOutput")
    if dbg:
        dbg_t = {
            "d_qw": nc.dram_tensor("d_qw", [128, S], f32, kind="ExternalOutput"),
            "d_kw": nc.dram_tensor("d_kw", [128, S], f32, kind="ExternalOutput"),
            "d_qkg": nc.dram_tensor("d_qkg", [128, 2 * S], f32, kind="ExternalOutput"),
            "d_v4": nc.dram_tensor("d_v4", [128, 16 * 260], f32, kind="ExternalOutput"),
            "d_cum0": nc.dram_tensor("d_cum0", [65, S], f32, kind="ExternalOutput"),
            "d_ul0": nc.dram_tensor("d_ul0", [65, S], f32, kind="ExternalOutput"),
            "d_uw0": nc.dram_tensor("d_uw0", [65, S], f32, kind="ExternalOutput"),
            "d_den": nc.dram_tensor("d_den", [128, S], f32, kind="ExternalOutput"),
            "d_recf": nc.dram_tensor("d_recf", [128, S], f32, kind="ExternalOutput"),
            "d_scl0": nc.dram_tensor("d_scl0", [128, S], f32, kind="ExternalOutput"),
        }

    NJ = 4          # number of 512-wide s blocks
    SB = 512        # s block width
    NT = 16         # number of 128-wide t chunks

    with tile.TileContext(nc) as tc:
        with (
            tc.tile_pool(name="sb", bufs=1) as sb,
            tc.tile_pool(name="sqp", bufs=5) as sqp,
            tc.tile_pool(name="exp", bufs=5) as exq,
            tc.tile_pool(name="stg", bufs=3) as stg,
            tc.tile_pool(name="bct", bufs=2) as bct,
            tc.tile_pool(name="psA", bufs=2, space="PSUM") as psA,
            tc.tile_pool(name="psAcc", bufs=2, space="PSUM") as psAcc,
            tc.tile_pool(name="psO", bufs=2, space="PSUM") as psO,
        ):
            # ---------------- persistent SBUF tiles ----------------
            ht_sb = sb.tile([128, 8, S], bf16, name="ht_sb")
            wqk_sb = sb.tile([128, 8, 384], bf16, name="wqk_sb")
            wv_sb = sb.tile([128, 8, 256], bf16, name="wv_sb")
            wo_sb = sb.tile([128, 2, 1024], bf16, name="wo_sb")
            msk_sb = sb.tile([128, 256], bf16, name="msk_sb")
            # qkg: lin q/k with ones row; h0 rows 0:17, h1 rows 32:49;
            # free index 0 = q, 1 = k
            qkg_sb = sb.tile([128, 2, S], bf16, name="qkg_sb")
            qw_sb = sb.tile([128, S], bf16, name="qw_sb")
            kw_sb = sb.tile([128, S], bf16, name="kw_sb")
            v4_sb = sb.tile([128, NT, 260], bf16, name="v4_sb")
            cum_sb = [sb.tile([65, S], f32, name=f"cum{h}_sb") for h in range(2)]
            ul_sb = [sb.tile([65, S], f32, name=f"ul{h}_sb") for h in range(2)]
            uw_sb = [sb.tile([65, S], f32, name=f"uw{h}_sb") for h in range(2)]
            scl_sb = [sb.tile([128, S], bf16, name=f"scl{h}_sb") for h in range(2)]
            den_sb = sb.tile([128, S], f32, name="den_sb")
            recf_sb = sb.tile([128, S], f32, name="recf_sb")
            recr_sb = sb.tile([1, 4, SB], f32, name="recr_sb")

            v4r = v4_sb.rearrange("p s (g x) -> p s g x", x=65)

            # ---------------- load inputs ----------------
            for k in range(8):
                nc.sync.dma_start(
                    out=ht_sb[:, k, 0:1024], in_=ht_d[ts(k, 128), 0:1024]
                )
                nc.sync.dma_start(out=wqk_sb[:, k, :], in_=wqk_d[ts(k, 128), :])
                nc.sync.dma_start(out=wv_sb[:, k, :], in_=wv_d[ts(k, 128), :])
            for k in range(8):
                nc.sync.dma_start(
                    out=ht_sb[:, k, 1024:2048], in_=ht_d[ts(k, 128), 1024:2048]
                )
            for k in range(2):
                nc.sync.dma_start(out=wo_sb[:, k, :], in_=wo_d[ts(k, 128), :])
            nc.sync.dma_start(out=msk_sb[:, :], in_=msk_d[:, :])
            # constant columns of v4: 0.5 for lin heads, 1.0 for win heads
            nc.gpsimd.memset(v4r[:, :, 0:2, 64], 0.5)
            nc.gpsimd.memset(v4r[:, :, 2:4, 64], 1.0)
            # garbage rows of den/recf must stay finite and initialized
            nc.gpsimd.memset(den_sb[:, :], 1.0)
            nc.gpsimd.memset(recf_sb[:, :], 1.0)

            # ---------------- phase 1a: q/k projections (paired j) ----------
            # wqk columns: [qw_a(64) qw_b(64) | kw_a(64) kw_b(64) |
            #   qlin_a@256 qlin_b@288 klin_a@320 klin_b@352 (16 each)]
            for jp in range(2):
                for blk in range(3):
                    c0 = blk * 128
                    js2 = ts(jp, 1024)
                    pp = psA.tile([128, 1024], f32, name="pp", tag="psA")
                    for jh in range(2):
                        for k in range(8):
                            nc.tensor.matmul(
                                pp[:, ts(jh, SB)],
                                lhsT=wqk_sb[:, k, c0 : c0 + 128],
                                rhs=ht_sb[:, k, ts(2 * jp + jh, SB)],
                                start=(k == 0),
                                stop=(k == 7),
                            )
                    if blk == 0:
                        nc.scalar.copy(out=qw_sb[:, js2], in_=pp[:, :])
                    elif blk == 1:
                        nc.scalar.copy(out=kw_sb[:, js2], in_=pp[:, :])
                    else:
                        nc.vector.tensor_copy(
                            out=qkg_sb[0:48, 0, js2], in_=pp[0:48, :]
                        )
                        nc.vector.tensor_copy(
                            out=qkg_sb[0:48, 1, js2], in_=pp[64:112, :]
                        )
                        # restore ones rows clobbered by the 48-row copies
                        for qk in range(2):
                            for r in (16, 48):
                                nc.sync.dma_start(
                                    out=qkg_sb[r : r + 1, qk, js2],
                                    in_=orow_d[0:1, js2],
                                )

            # ---------------- phase 1b: v projections (paired st) -----------
            for sp_ in range(8):
                st0 = 2 * sp_
                pv = psA.tile([128, 1024], f32, name="pv", tag="psA")
                for sh in range(2):
                    for k in range(8):
                        nc.tensor.matmul(
                            pv[:, sh * 256 : sh * 256 + 256],
                            lhsT=ht_sb[:, k, ts(st0 + sh, 128)],
                            rhs=wv_sb[:, k, :],
                            start=(k == 0),
                            stop=(k == 7),
                        )
                nc.vector.tensor_copy(
                    out=v4r[:, st0 : st0 + 2, :, 0:64],
                    in_=pv[:, 0:512].rearrange("p (s g x) -> p s g x", s=2, x=64),
                )

            # ---------------- phase 1c: CUM (cumulative v-half sums) -------
            # cum[h][d, s] = sum_{t<=s} vlin_half[t, d]; row 64 = 0.5*(s+1)
            for h in range(2):
                lin_sl = slice(65 * h, 65 * h + 65)
                for sj in range(NT):
                    icp = psA.tile([128, 1024], f32, name="icp", tag="psA")
                    nc.tensor.matmul(
                        icp[0:65, 0:128],
                        lhsT=v4_sb[:, sj, lin_sl],
                        rhs=msk_sb[:, 0:128],
                        start=True,
                        stop=True,
                    )
                    if sj == 0:
                        nc.vector.tensor_scalar(
                            out=cum_sb[h][:, 0:128],
                            in0=icp[0:65, 0:128],
                            scalar1=0.0,
                            scalar2=None,
                            op0=ADD,
                        )
                    else:
                        nc.vector.tensor_scalar(
                            out=cum_sb[h][:, ts(sj, 128)],
                            in0=icp[0:65, 0:128],
                            scalar1=cum_sb[h][:, sj * 128 - 1 : sj * 128],
                            scalar2=None,
                            op0=ADD,
                        )

            # ---------------- phase 2 + pipelined tails ----------------
            # Emitted as interleaved blocks: lin/win alternate per t-chunk,
            # accumulation matmuls trail their producer by 2 blocks, and the
            # previous j-block's tail pieces are spliced into the stream so
            # no engine queues long dependency stalls.
            def phase2_blocks(j):
                js = ts(j, SB)
                lim_l = 4 * j + 4
                lim_w = min(16, 4 * j + 6)
                st_ = {}

                def begin():
                    st_["qkv"] = [
                        psAcc.tile([65, SB], f32, name=f"qkv{h}", tag="acc")
                        for h in range(2)
                    ]
                    st_["sq"] = {}

                def lin_blk(ti):
                    mp = psA.tile([128, 1024], f32, name="mp", tag="psA")
                    for h, (p0, p1) in enumerate(((0, 17), (32, 49))):
                        nc.tensor.matmul(
                            mp[:, ts(h, SB)],
                            lhsT=qkg_sb[p0:p1, 1, ts(ti, 128)],
                            rhs=qkg_sb[p0:p1, 0, js],
                            start=True,
                            stop=True,
                        )
                    sq = sqp.tile([128, 1024], bf16, name="sq", tag="sq")
                    col0 = max(0, ti - 4 * j) * 128
                    sqr = sq.rearrange("p (g x) -> p g x", x=SB)
                    mpr = mp.rearrange("p (g x) -> p g x", x=SB)
                    if col0:
                        nc.gpsimd.memset(sqr[:, :, 0:col0], 0.0)
                    nc.scalar.activation(
                        sqr[:, :, col0:SB], mpr[:, :, col0:SB], Square
                    )
                    sd = ti - 4 * j
                    if 0 <= sd <= 3:
                        for h in range(2):
                            dsl = slice(h * SB + sd * 128, h * SB + (sd + 1) * 128)
                            nc.gpsimd.tensor_tensor(
                                out=sq[:, dsl], in0=sq[:, dsl],
                                in1=msk_sb[:, 0:128], op=MULT,
                            )
                    st_["sq"][ti] = sq

                def lin_acc(ti):
                    sqt = st_["sq"].pop(ti)
                    for h in range(2):
                        nc.tensor.matmul(
                            st_["qkv"][h][:, :],
                            lhsT=v4_sb[:, ti, slice(65 * h, 65 * h + 65)],
                            rhs=sqt[:, ts(h, SB)],
                            start=(ti == 0),
                            stop=(ti == lim_l - 1),
                            skip_group_check=True,
                        )

                def lin_end():
                    for h in range(2):
                        nc.vector.tensor_tensor(
                            out=ul_sb[h][:, js],
                            in0=st_["qkv"][h][0:65, :],
                            in1=cum_sb[h][:, js],
                            op=ADD,
                        )
                        nc.sync.dma_start(
                            out=den_sb[32 * h : 32 * h + 1, js],
                            in_=ul_sb[h][64:65, js],
                        )

                def win_begin():
                    st_["nt"] = [
                        psAcc.tile([65, SB], f32, name=f"nt{h}", tag="acc")
                        for h in range(2)
                    ]
                    st_["ex"] = {}

                def win_blk(ti):
                    sp = psA.tile([128, 1024], f32, name="sp", tag="psA")
                    for h in range(2):
                        hsl = slice(64 * h, 64 * h + 64)
                        nc.tensor.matmul(
                            sp[:, ts(h, SB)],
                            lhsT=kw_sb[hsl, ts(ti, 128)],
                            rhs=qw_sb[hsl, js],
                            start=True,
                            stop=True,
                        )
                    ex = exq.tile([128, 1024], bf16, name="ex", tag="ex")
                    col0 = max(0, ti - 2 - 4 * j) * 128
                    exr = ex.rearrange("p (g x) -> p g x", x=SB)
                    spr = sp.rearrange("p (g x) -> p g x", x=SB)
                    if col0:
                        nc.gpsimd.memset(exr[:, :, 0:col0], 0.0)
                    nc.scalar.activation(
                        exr[:, :, col0:SB], spr[:, :, col0:SB], Exp, scale=0.125
                    )
                    sd = ti - 2 - 4 * j
                    if 0 <= sd <= 3:
                        for h in range(2):
                            dsl = slice(h * SB + sd * 128, h * SB + (sd + 1) * 128)
                            nc.gpsimd.tensor_tensor(
                                out=ex[:, dsl], in0=ex[:, dsl],
                                in1=msk_sb[:, 128:256], op=MULT,
                            )
                    st_["ex"][ti] = ex

                def win_acc(ti):
                    ext = st_["ex"].pop(ti)
                    for h in range(2):
                        nc.tensor.matmul(
                            st_["nt"][h][:, :],
                            lhsT=v4_sb[:, ti, slice(130 + 65 * h, 195 + 65 * h)],
                            rhs=ext[:, ts(h, SB)],
                            start=(ti == 0),
                            stop=(ti == lim_w - 1),
                            skip_group_check=True,
                        )

                def win_end():
                    for h in range(2):
                        nc.vector.tensor_copy(
                            out=uw_sb[h][:, js], in_=st_["nt"][h][0:65, :]
                        )
                        nc.sync.dma_start(
                            out=den_sb[64 + 32 * h : 65 + 32 * h, js],
                            in_=uw_sb[h][64:65, js],
                        )

                # block list: lin chain then win chain, lag-2 accumulation
                blocks = [begin]
                for ti in range(lim_l):
                    blocks.append(lambda ti=ti: lin_blk(ti))
                    if ti >= 2:
                        blocks.append(lambda ti=ti - 2: lin_acc(ti))
                blocks.append(lambda: lin_acc(lim_l - 2))
                blocks.append(lambda: lin_acc(lim_l - 1))
                blocks.append(lin_end)
                blocks.append("WIN_START")
                blocks.append(win_begin)
                for ti in range(lim_w):
                    blocks.append(lambda ti=ti: win_blk(ti))
                    if ti >= 2:
                        blocks.append(lambda ti=ti - 2: win_acc(ti))
                blocks.append(lambda: win_acc(lim_w - 2))
                blocks.append(lambda: win_acc(lim_w - 1))
                blocks.append(win_end)
                return blocks

            recf4v = None

            def tail_lin_pieces(j):
                js = ts(j, SB)
                pieces = []

                def lin_recip():
                    nc.vector.tensor_scalar(
                        out=recf_sb[0:33, js], in0=den_sb[0:33, js],
                        scalar1=EPS, scalar2=None, op0=ADD,
                    )
                    nc.vector.reciprocal(
                        out=recf_sb[0:33, js], in_=recf_sb[0:33, js]
                    )
                    rv = recf_sb.rearrange("(a b) f -> a b f", b=32)
                    nc.sync.dma_start(
                        out=recr_sb[0:1, 0:2, :], in_=rv[0:2, 0, js]
                    )

                pieces.append(lin_recip)

                def scale(h):
                    bc = bct.tile([64, SB], f32, name="bc", tag="bc")
                    nc.gpsimd.partition_broadcast(bc[:, :], recr_sb[0:1, h, :])
                    nc.vector.tensor_tensor(
                        out=scl_sb[h][0:64, js],
                        in0=ul_sb[h][0:64, js],
                        in1=bc[:, :],
                        op=MULT,
                    )

                for h in range(2):
                    pieces.append(lambda h=h: scale(h))
                return pieces

            def tail_win_pieces(j):
                js = ts(j, SB)
                pieces = []

                def win_recip():
                    nc.vector.tensor_scalar(
                        out=recf_sb[64:97, js], in0=den_sb[64:97, js],
                        scalar1=EPS, scalar2=None, op0=ADD,
                    )
                    nc.vector.reciprocal(
                        out=recf_sb[64:97, js], in_=recf_sb[64:97, js]
                    )
                    rv = recf_sb.rearrange("(a b) f -> a b f", b=32)
                    nc.sync.dma_start(
                        out=recr_sb[0:1, 2:4, :], in_=rv[2:4, 0, js]
                    )

                pieces.append(win_recip)

                def scale(h):
                    bc = bct.tile([64, SB], f32, name="bc", tag="bc")
                    nc.gpsimd.partition_broadcast(bc[:, :], recr_sb[0:1, 2 + h, :])
                    nc.vector.tensor_tensor(
                        out=scl_sb[h][64:128, js],
                        in0=uw_sb[h][0:64, js],
                        in1=bc[:, :],
                        op=MULT,
                    )

                for h in range(2):
                    pieces.append(lambda h=h: scale(h))

                def final(st, nb):
                    po = psO.tile([128, SB], f32, name="po", tag="po")
                    for h in range(2):
                        nc.tensor.matmul(
                            po[:, :],
                            lhsT=scl_sb[h][:, ts(st, 128)],
                            rhs=wo_sb[:, h, ts(nb, SB)],
                            start=(h == 0),
                            stop=(h == 1),
                            skip_group_check=True,
                        )
                    so = stg.tile([128, SB], f32, name="so", tag="so")
                    if (st + nb) % 2 == 0:
                        nc.vector.tensor_copy(out=so[:, :], in_=po[:, :])
                    else:
                        nc.scalar.copy(out=so[:, :], in_=po[:, :])
                    nc.sync.dma_start(
                        out=out_d[ts(st, 128), ts(nb, SB)], in_=so[:, :]
                    )

                for st in range(4 * j, 4 * j + 4):
                    for nb in range(2):
                        pieces.append(lambda st=st, nb=nb: final(st, nb))
                return pieces

            def interleave(blocks, pieces):
                wstart = blocks.index("WIN_START") if "WIN_START" in blocks else 0
                blocks = [b for b in blocks if b != "WIN_START"]
                if not pieces:
                    for b in blocks:
                        b()
                    return
                nwin = len(blocks) - wstart
                stride = max(1, nwin // (len(pieces) + 1))
                pi = 0
                for i, b in enumerate(blocks):
                    b()
                    if i >= wstart and (i - wstart + 1) % stride == 0 and pi < len(pieces):
                        pieces[pi]()
                        pi += 1
                while pi < len(pieces):
                    pieces[pi]()
                    pi += 1

            interleave(phase2_blocks(0), tail_lin_pieces(0))
            interleave(phase2_blocks(1), tail_win_pieces(0) + tail_lin_pieces(1))
            interleave(phase2_blocks(2), tail_win_pieces(1) + tail_lin_pieces(2))
            interleave(phase2_blocks(3), tail_win_pieces(2) + tail_lin_pieces(3))
            for p in tail_win_pieces(3):
                p()

    nc.compile()
    return nc


def _prep_inputs(inputs):
    """Host-side sharding/packing. Returns per-core input maps."""
    h = np.asarray(inputs["hidden_states"], np.float32).reshape(S, D)
    ht = np.ascontiguousarray(h.T).astype(BF)

    lin_Wq = np.asarray(inputs["lin_Wq"], np.float32)
    lin_Wk = np.asarray(inputs["lin_Wk"], np.float32)
    lin_Wv = np.asarray(inputs["lin_Wv"], np.float32)
    lin_Wo = np.asarray(inputs["lin_Wo"], np.float32)
    win_Wq = np.asarray(inputs["win_Wq"], np.float32)
    win_Wk = np.asarray(inputs["win_Wk"], np.float32)
    win_Wv = np.asarray(inputs["win_Wv"], np.float32)
    win_Wo = np.asarray(inputs["win_Wo"], np.float32)

    # constant mask tiles
    p = np.arange(128)[:, None]
    f = np.arange(128)[None, :]
    msk = np.zeros((128, 256), np.float32)
    msk[:, 0:128] = (p <= f)          # lin diag mask (t <= s)
    msk[:, 128:256] = (p < f)         # win partial mask (t < s)

    in_maps = []
    for c in range(NCORES):
        a, b = 2 * c, 2 * c + 1
        wqk = np.zeros((D, 384), np.float32)
        wqk[:, 0:64] = win_Wq[:, a * HD : (a + 1) * HD]
        wqk[:, 64:128] = win_Wq[:, b * HD : (b + 1) * HD]
        wqk[:, 128:192] = win_Wk[:, a * HD : (a + 1) * HD]
        wqk[:, 192:256] = win_Wk[:, b * HD : (b + 1) * HD]
        wqk[:, 256:272] = lin_Wq[:, a * FD : (a + 1) * FD] * 0.5
        wqk[:, 288:304] = lin_Wq[:, b * FD : (b + 1) * FD] * 0.5
        wqk[:, 320:336] = lin_Wk[:, a * FD : (a + 1) * FD] * 0.5
        wqk[:, 352:368] = lin_Wk[:, b * FD : (b + 1) * FD] * 0.5
        wv = np.zeros((D, 256), np.float32)
        wv[:, 0:64] = lin_Wv[:, a * HD : (a + 1) * HD] * 0.5
        wv[:, 64:128] = lin_Wv[:, b * HD : (b + 1) * HD] * 0.5
        wv[:, 128:192] = win_Wv[:, a * HD : (a + 1) * HD]
        wv[:, 192:256] = win_Wv[:, b * HD : (b + 1) * HD]
        wo = np.zeros((256, D), np.float32)
        wo[0:64] = lin_Wo[a * HD : (a + 1) * HD]
        wo[64:128] = win_Wo[a * HD : (a + 1) * HD]
        wo[128:192] = lin_Wo[b * HD : (b + 1) * HD]
        wo[192:256] = win_Wo[b * HD : (b + 1) * HD]
        in_maps.append(
            {
                "ht": ht,
                "wqk": wqk.astype(BF),
                "wv": wv.astype(BF),
                "wo": wo.astype(BF),
                "msk": msk.astype(BF),
                "orow": np.ones((1, S), np.float32).astype(BF),
            }
        )
    return in_maps


def kernel(**inputs) -> np.ndarray:
    from concourse.bass_utils import run_bass_kernel_spmd

    if "nc" not in _CACHE:
        _CACHE["nc"] = _build_nc()
    nc = _CACHE["nc"]
    in_maps = _prep_inputs(inputs)
    res = run_bass_kernel_spmd(nc, in_maps, core_ids=list(range(NCORES)))
    out = np.zeros((S, D), np.float32)
    for r in res.results:
        out += r["out"]
    return out.reshape(1, S, D)


if __name__ == "__main__":
    nc = _build_nc()
    print("built ok")


# revision 28
# speedup vs baseline: 1.1048x; 1.0105x over previous
"""Trainium2 Bass kernel for nn_Based_40630390620259 (sparse_attention).

Architecture ("Based"-style): linear (Taylor feature-map) attention +
windowed softmax attention, 16 heads, S=2048, D=1024.

Math identities used (verified against the reference to 1e-6):
  - Taylor feature map inner product collapses:
        qf.kf = 1 + (q.k)/4 + (q.k)^2/32 = 0.5 + 0.5*(1 + q.k/4)^2
    so the 273-dim feature space is never materialized. With Wq,Wk scaled
    by 0.5 on the host and a constant ones-row appended to q/k (K=17
    matmul), the PE produces m'' = 1 + q.k/4 directly; sq = m''^2 on DVE.
  - The 0.5 factor is folded into the V projection weights; the +0.5
    constant term contributes a causal cumulative sum CUM of the
    (0.5-scaled) v rows, computed with 16 N=128 matmuls against an
    upper-triangular ones block + a recursive per-partition scalar-add.
    CUM row 64 (from the 0.5-constant column) equals 0.5*(s+1), which is
    exactly the constant-term part of the reference denominator.
  - win path: scores^T computed as [t,s] tiles; softmax denominator via a
    ones-column in V'; division deferred through the output projection via
    a gpsimd partition_broadcast of the reciprocal row.

Sharding: tensor-parallel over heads, 2 heads per core, 8 cores. Each core
produces a partial [S, D] output (its heads' contribution); the host sums.
Both heads are processed per t-chunk with 2-way row-strip packing (lin at
array rows 0/32, win at rows 0/64) writing the two halves of paired
[128,1024] PSUM tiles, so elementwise ops cover both heads in one
instruction.

Self-contained: only imports concourse/* from the environment.
"""

import numpy as np
import ml_dtypes

S = 2048
D = 1024
H = 16
FD = 16
HD = 64
W = 256
EPS = 1e-9
NCORES = 8

BF = ml_dtypes.bfloat16

_CACHE = {}


def _build_nc(dbg=False):
    import concourse.bass as bass
    import concourse.mybir as mybir
    import concourse.tile as tile
    from concourse import bacc
    from concourse.bass import ts

    f32 = mybir.dt.float32
    bf16 = mybir.dt.bfloat16
    MULT = mybir.AluOpType.mult
    ADD = mybir.AluOpType.add
    Exp = mybir.ActivationFunctionType.Exp
    Square = mybir.ActivationFunctionType.Square

    nc = bacc.Bacc("TRN2", target_bir_lowering=False)

    ht_d = nc.dram_tensor("ht", [D, S], bf16, kind="ExternalInput")
    wqk_d = nc.dram_tensor("wqk", [D, 384], bf16, kind="ExternalInput")
    wv_d = nc.dram_tensor("wv", [D, 256], bf16, kind="ExternalInput")
    wo_d = nc.dram_tensor("wo", [256, D], bf16, kind="ExternalInput")
    msk_d = nc.dram_tensor("msk", [128, 256], bf16, kind="ExternalInput")
    orow_d = nc.dram_tensor("orow", [1, S], bf16, kind="ExternalInput")
    out_d = nc.dram_tensor("out", [S, D], f32, kind="ExternalOutput")
    if dbg:
        dbg_t = {
            "d_qw": nc.dram_tensor("d_qw", [128, S], f32, kind="ExternalOutput"),
            "d_kw": nc.dram_tensor("d_kw", [128, S], f32, kind="ExternalOutput"),
            "d_qkg": nc.dram_tensor("d_qkg", [128, 2 * S], f32, kind="ExternalOutput"),
            "d_v4": nc.dram_tensor("d_v4", [128, 16 * 260], f32, kind="ExternalOutput"),
            "d_cum0": nc.dram_tensor("d_cum0", [65, S], f32, kind="ExternalOutput"),
            "d_ul0": nc.dram_tensor("d_ul0", [65, S], f32, kind="ExternalOutput"),
            "d_uw0": nc.dram_tensor("d_uw0", [65, S], f32, kind="ExternalOutput"),
            "d_den": nc.dram_tensor("d_den", [128, S], f32, kind="ExternalOutput"),
            "d_recf": nc.dram_tensor("d_recf", [128, S], f32, kind="ExternalOutput"),
            "d_scl0": nc.dram_tensor("d_scl0", [128, S], f32, kind="ExternalOutput"),
        }

    NJ = 4          # number of 512-wide s blocks
    SB = 512        # s block width
    NT = 16         # number of 128-wide t chunks

    with tile.TileContext(nc) as tc:
        with (
            tc.tile_pool(name="sb", bufs=1) as sb,
            tc.tile_pool(name="sqp", bufs=5) as sqp,
            tc.tile_pool(name="exp", bufs=5) as exq,
            tc.tile_pool(name="stg", bufs=3) as stg,
            tc.tile_pool(name="bct", bufs=2) as bct,
            tc.tile_pool(name="psA", bufs=2, space="PSUM") as psA,
            tc.tile_pool(name="psAcc", bufs=2, space="PSUM") as psAcc,
            tc.tile_pool(name="psO", bufs=2, space="PSUM") as psO,
        ):
            # ---------------- persistent SBUF tiles ----------------
            ht_sb = sb.tile([128, 8, S], bf16, name="ht_sb")
            wqk_sb = sb.tile([128, 8, 384], bf16, name="wqk_sb")
            wv_sb = sb.tile([128, 8, 256], bf16, name="wv_sb")
            wo_sb = sb.tile([128, 2, 1024], bf16, name="wo_sb")
            msk_sb = sb.tile([128, 256], bf16, name="msk_sb")
            # qkg: lin q/k with ones row; h0 rows 0:17, h1 rows 32:49;
            # free index 0 = q, 1 = k
            qkg_sb = sb.tile([128, 2, S], bf16, name="qkg_sb")
            qw_sb = sb.tile([128, S], bf16, name="qw_sb")
            kw_sb = sb.tile([128, S], bf16, name="kw_sb")
            v4_sb = sb.tile([128, NT, 260], bf16, name="v4_sb")
            cum_sb = [sb.tile([65, S], f32, name=f"cum{h}_sb") for h in range(2)]
            ul_sb = [sb.tile([65, S], f32, name=f"ul{h}_sb") for h in range(2)]
            uw_sb = [sb.tile([65, S], f32, name=f"uw{h}_sb") for h in range(2)]
            scl_sb = [sb.tile([128, S], bf16, name=f"scl{h}_sb") for h in range(2)]
            den_sb = sb.tile([128, S], f32, name="den_sb")
            recf_sb = sb.tile([128, S], f32, name="recf_sb")
            recr_sb = sb.tile([1, 4, SB], f32, name="recr_sb")

            v4r = v4_sb.rearrange("p s (g x) -> p s g x", x=65)

            # ---------------- load inputs ----------------
            for k in range(8):
                nc.sync.dma_start(
                    out=ht_sb[:, k, 0:1024], in_=ht_d[ts(k, 128), 0:1024]
                )
                nc.sync.dma_start(out=wqk_sb[:, k, :], in_=wqk_d[ts(k, 128), :])
                nc.sync.dma_start(out=wv_sb[:, k, :], in_=wv_d[ts(k, 128), :])
            for k in range(8):
                nc.sync.dma_start(
                    out=ht_sb[:, k, 1024:2048], in_=ht_d[ts(k, 128), 1024:2048]
                )
            for k in range(2):
                nc.sync.dma_start(out=wo_sb[:, k, :], in_=wo_d[ts(k, 128), :])
            nc.sync.dma_start(out=msk_sb[:, :], in_=msk_d[:, :])
            # constant columns of v4: 0.5 for lin heads, 1.0 for win heads
            nc.gpsimd.memset(v4r[:, :, 0:2, 64], 0.5)
            nc.gpsimd.memset(v4r[:, :, 2:4, 64], 1.0)
            # garbage rows of den/recf must stay finite and initialized
            nc.gpsimd.memset(den_sb[:, :], 1.0)
            nc.gpsimd.memset(recf_sb[:, :], 1.0)

            # ---------------- phase 1a: q/k projections (paired j) ----------
            # wqk columns: [qw_a(64) qw_b(64) | kw_a(64) kw_b(64) |
            #   qlin_a@256 qlin_b@288 klin_a@320 klin_b@352 (16 each)]
            for jp in range(2):
                for blk in range(3):
                    c0 = blk * 128
                    js2 = ts(jp, 1024)
                    pp = psA.tile([128, 1024], f32, name="pp", tag="psA")
                    for jh in range(2):
                        for k in range(8):
                            nc.tensor.matmul(
                                pp[:, ts(jh, SB)],
                                lhsT=wqk_sb[:, k, c0 : c0 + 128],
                                rhs=ht_sb[:, k, ts(2 * jp + jh, SB)],
                                start=(k == 0),
                                stop=(k == 7),
                            )
                    if blk == 0:
                        nc.scalar.copy(out=qw_sb[:, js2], in_=pp[:, :])
                    elif blk == 1:
                        nc.scalar.copy(out=kw_sb[:, js2], in_=pp[:, :])
                    else:
                        nc.vector.tensor_copy(
                            out=qkg_sb[0:48, 0, js2], in_=pp[0:48, :]
                        )
                        nc.vector.tensor_copy(
                            out=qkg_sb[0:48, 1, js2], in_=pp[64:112, :]
                        )
                        # restore ones rows clobbered by the 48-row copies
                        for qk in range(2):
                            for r in (16, 48):
                                nc.sync.dma_start(
                                    out=qkg_sb[r : r + 1, qk, js2],
                                    in_=orow_d[0:1, js2],
                                )

            # ---------------- phase 1b: v projections (paired st) -----------
            for sp_ in range(8):
                st0 = 2 * sp_
                pv = psA.tile([128, 1024], f32, name="pv", tag="psA")
                for sh in range(2):
                    for k in range(8):
                        nc.tensor.matmul(
                            pv[:, sh * 256 : sh * 256 + 256],
                            lhsT=ht_sb[:, k, ts(st0 + sh, 128)],
                            rhs=wv_sb[:, k, :],
                            start=(k == 0),
                            stop=(k == 7),
                        )
                nc.vector.tensor_copy(
                    out=v4r[:, st0 : st0 + 2, :, 0:64],
                    in_=pv[:, 0:512].rearrange("p (s g x) -> p s g x", s=2, x=64),
                )

            # ---------------- phase 1c: CUM (cumulative v-half sums) -------
            # cum[h][d, s] = sum_{t<=s} vlin_half[t, d]; row 64 = 0.5*(s+1)
            for h in range(2):
                lin_sl = slice(65 * h, 65 * h + 65)
                for sj in range(NT):
                    icp = psA.tile([128, 1024], f32, name="icp", tag="psA")
                    nc.tensor.matmul(
                        icp[0:65, 0:128],
                        lhsT=v4_sb[:, sj, lin_sl],
                        rhs=msk_sb[:, 0:128],
                        start=True,
                        stop=True,
                    )
                    if sj == 0:
                        nc.vector.tensor_scalar(
                            out=cum_sb[h][:, 0:128],
                            in0=icp[0:65, 0:128],
                            scalar1=0.0,
                            scalar2=None,
                            op0=ADD,
                        )
                    else:
                        nc.vector.tensor_scalar(
                            out=cum_sb[h][:, ts(sj, 128)],
                            in0=icp[0:65, 0:128],
                            scalar1=cum_sb[h][:, sj * 128 - 1 : sj * 128],
                            scalar2=None,
                            op0=ADD,
                        )

            # ---------------- phase 2 + pipelined tails ----------------
            # Emitted as interleaved blocks: lin/win alternate per t-chunk,
            # accumulation matmuls trail their producer by 2 blocks, and the
            # previous j-block's tail pieces are spliced into the stream so
            # no engine queues long dependency stalls.
            def phase2_blocks(j):
                js = ts(j, SB)
                lim_l = 4 * j + 4
                lim_w = min(16, 4 * j + 6)
                st_ = {}

                def begin():
                    st_["qkv"] = [
                        psAcc.tile([65, SB], f32, name=f"qkv{h}", tag="acc")
                        for h in range(2)
                    ]
                    st_["sq"] = {}

                def lin_blk(ti):
                    mp = psA.tile([128, 1024], f32, name="mp", tag="psA")
                    for h, (p0, p1) in enumerate(((0, 17), (32, 49))):
                        nc.tensor.matmul(
                            mp[:, ts(h, SB)],
                            lhsT=qkg_sb[p0:p1, 1, ts(ti, 128)],
                            rhs=qkg_sb[p0:p1, 0, js],
                            start=True,
                            stop=True,
                        )
                    sq = sqp.tile([128, 1024], bf16, name="sq", tag="sq")
                    col0 = max(0, ti - 4 * j) * 128
                    sqr = sq.rearrange("p (g x) -> p g x", x=SB)
                    mpr = mp.rearrange("p (g x) -> p g x", x=SB)
                    if col0:
                        nc.gpsimd.memset(sqr[:, :, 0:col0], 0.0)
                    nc.scalar.activation(
                        sqr[:, :, col0:SB], mpr[:, :, col0:SB], Square
                    )
                    sd = ti - 4 * j
                    if 0 <= sd <= 3:
                        for h in range(2):
                            dsl = slice(h * SB + sd * 128, h * SB + (sd + 1) * 128)
                            nc.gpsimd.tensor_tensor(
                                out=sq[:, dsl], in0=sq[:, dsl],
                                in1=msk_sb[:, 0:128], op=MULT,
                            )
                    st_["sq"][ti] = sq

                def lin_acc(ti):
                    sqt = st_["sq"].pop(ti)
                    for h in range(2):
                        nc.tensor.matmul(
                            st_["qkv"][h][:, :],
                            lhsT=v4_sb[:, ti, slice(65 * h, 65 * h + 65)],
                            rhs=sqt[:, ts(h, SB)],
                            start=(ti == 0),
                            stop=(ti == lim_l - 1),
                            skip_group_check=True,
                        )

                def lin_end():
                    for h in range(2):
                        nc.vector.tensor_tensor(
                            out=ul_sb[h][:, js],
                            in0=st_["qkv"][h][0:65, :],
                            in1=cum_sb[h][:, js],
                            op=ADD,
                        )
                        nc.sync.dma_start(
                            out=den_sb[32 * h : 32 * h + 1, js],
                            in_=ul_sb[h][64:65, js],
                        )

                def win_begin():
                    st_["nt"] = [
                        psAcc.tile([65, SB], f32, name=f"nt{h}", tag="acc")
                        for h in range(2)
                    ]
                    st_["ex"] = {}

                def win_blk(ti):
                    sp = psA.tile([128, 1024], f32, name="sp", tag="psA")
                    for h in range(2):
                        hsl = slice(64 * h, 64 * h + 64)
                        nc.tensor.matmul(
                            sp[:, ts(h, SB)],
                            lhsT=kw_sb[hsl, ts(ti, 128)],
                            rhs=qw_sb[hsl, js],
                            start=True,
                            stop=True,
                        )
                    ex = exq.tile([128, 1024], bf16, name="ex", tag="ex")
                    col0 = max(0, ti - 2 - 4 * j) * 128
                    exr = ex.rearrange("p (g x) -> p g x", x=SB)
                    spr = sp.rearrange("p (g x) -> p g x", x=SB)
                    if col0:
                        nc.gpsimd.memset(exr[:, :, 0:col0], 0.0)
                    nc.scalar.activation(
                        exr[:, :, col0:SB], spr[:, :, col0:SB], Exp, scale=0.125
                    )
                    sd = ti - 2 - 4 * j
                    if 0 <= sd <= 3:
                        for h in range(2):
                            dsl = slice(h * SB + sd * 128, h * SB + (sd + 1) * 128)
                            nc.gpsimd.tensor_tensor(
                                out=ex[:, dsl], in0=ex[:, dsl],
                                in1=msk_sb[:, 128:256], op=MULT,
                            )
                    st_["ex"][ti] = ex

                def win_acc(ti):
                    ext = st_["ex"].pop(ti)
                    for h in range(2):
                        nc.tensor.matmul(
                            st_["nt"][h][:, :],
                            lhsT=v4_sb[:, ti, slice(130 + 65 * h, 195 + 65 * h)],
                            rhs=ext[:, ts(h, SB)],
                            start=(ti == 0),
                            stop=(ti == lim_w - 1),
                            skip_group_check=True,
                        )

                def win_end():
                    for h in range(2):
                        nc.vector.tensor_copy(
                            out=uw_sb[h][:, js], in_=st_["nt"][h][0:65, :]
                        )
                        nc.sync.dma_start(
                            out=den_sb[64 + 32 * h : 65 + 32 * h, js],
                            in_=uw_sb[h][64:65, js],
                        )

                # block list: lin chain then win chain, lag-2 accumulation
                blocks = [begin]
                for ti in range(lim_l):
                    blocks.append(lambda ti=ti: lin_blk(ti))
                    if ti >= 2:
                        blocks.append(lambda ti=ti - 2: lin_acc(ti))
                blocks.append(lambda: lin_acc(lim_l - 2))
                blocks.append(lambda: lin_acc(lim_l - 1))
                blocks.append(lin_end)
                blocks.append("WIN_START")
                blocks.append(win_begin)
                for ti in range(lim_w):
                    blocks.append(lambda ti=ti: win_blk(ti))
                    if ti >= 2:
                        blocks.append(lambda ti=ti - 2: win_acc(ti))
                blocks.append(lambda: win_acc(lim_w - 2))
                blocks.append(lambda: win_acc(lim_w - 1))
                blocks.append(win_end)
                return blocks

            recf4v = None

            def tail_lin_pieces(j):
                js = ts(j, SB)
                pieces = []

                def lin_recip():
                    nc.vector.tensor_scalar(
                        out=recf_sb[0:33, js], in0=den_sb[0:33, js],
                        scalar1=EPS, scalar2=None, op0=ADD,
                    )
                    nc.vector.reciprocal(
                        out=recf_sb[0:33, js], in_=recf_sb[0:33, js]
                    )
                    rv = recf_sb.rearrange("(a b) f -> a b f", b=32)
                    nc.sync.dma_start(
                        out=recr_sb[0:1, 0:2, :], in_=rv[0:2, 0, js]
                    )

                pieces.append(lin_recip)

                def scale(h):
                    bc = bct.tile([64, SB], f32, name="bc", tag="bc")
                    nc.gpsimd.partition_broadcast(bc[:, :], recr_sb[0:1, h, :])
                    nc.vector.tensor_tensor(
                        out=scl_sb[h][0:64, js],
                        in0=ul_sb[h][0:64, js],
                        in1=bc[:, :],
                        op=MULT,
                    )

                for h in range(2):
                    pieces.append(lambda h=h: scale(h))
                return pieces

            def tail_win_pieces(j):
                js = ts(j, SB)
                pieces = []

                def win_recip():
                    nc.vector.tensor_scalar(
                        out=recf_sb[64:97, js], in0=den_sb[64:97, js],
                        scalar1=EPS, scalar2=None, op0=ADD,
                    )
                    nc.vector.reciprocal(
                        out=recf_sb[64:97, js], in_=recf_sb[64:97, js]
                    )
                    rv = recf_sb.rearrange("(a b) f -> a b f", b=32)
                    nc.sync.dma_start(
                        out=recr_sb[0:1, 2:4, :], in_=rv[2:4, 0, js]
                    )

                pieces.append(win_recip)

                def scale(h):
                    bc = bct.tile([64, SB], f32, name="bc", tag="bc")
                    nc.gpsimd.partition_broadcast(bc[:, :], recr_sb[0:1, 2 + h, :])
                    nc.vector.tensor_tensor(
                        out=scl_sb[h][64:128, js],
                        in0=uw_sb[h][0:64, js],
                        in1=bc[:, :],
                        op=MULT,
                    )

                for h in range(2):
                    pieces.append(lambda h=h: scale(h))

                def final(st, nb):
                    po = psO.tile([128, SB], f32, name="po", tag="po")
                    for h in range(2):
                        nc.tensor.matmul(
                            po[:, :],
                            lhsT=scl_sb[h][:, ts(st, 128)],
                            rhs=wo_sb[:, h, ts(nb, SB)],
                            start=(h == 0),
                            stop=(h == 1),
                            skip_group_check=True,
                        )
                    so = stg.tile([128, SB], f32, name="so", tag="so")
                    if (st + nb) % 2 == 0:
                        nc.vector.tensor_copy(out=so[:, :], in_=po[:, :])
                    else:
                        nc.scalar.copy(out=so[:, :], in_=po[:, :])
                    nc.sync.dma_start(
                        out=out_d[ts(st, 128), ts(nb, SB)], in_=so[:, :]
                    )

                for st in range(4 * j, 4 * j + 4):
                    for nb in range(2):
                        pieces.append(lambda st=st, nb=nb: final(st, nb))
                return pieces

            def interleave(blocks, pieces):
                wstart = blocks.index("WIN_START") if "WIN_START" in blocks else 0
                blocks = [b for b in blocks if b != "WIN_START"]
                if not pieces:
                    for b in blocks:
                        b()
                    return
                nwin = len(blocks) - wstart
                stride = max(1, nwin // (len(pieces) + 1))
                pi = 0
                for i, b in enumerate(blocks):
                    b()
                    if i >= wstart and (i - wstart + 1) % stride == 0 and pi < len(pieces):
                        pieces[pi]()
                        pi += 1
                while pi < len(pieces):
                    pieces[pi]()
                    pi += 1

            def run_phase(j, early, late):
                blocks = phase2_blocks(j)
                wstart = blocks.index("WIN_START")
                blocks = [b for b in blocks if b != "WIN_START"]
                nwin = len(blocks) - wstart
                stride = max(1, nwin // (len(early) + 1)) if early else 10**9
                pi = 0
                for i, b in enumerate(blocks):
                    b()
                    if i >= wstart and (i - wstart + 1) % stride == 0 and pi < len(early):
                        early[pi]()
                        pi += 1
                while pi < len(early):
                    early[pi]()
                    pi += 1
                for p in late:
                    p()

            tw = {}
            run_phase(0, tail_lin_pieces(0), [])
            tw[0] = tail_win_pieces(0)
            run_phase(1, tw[0][:3] + tail_lin_pieces(1), tw[0][3:])
            tw[1] = tail_win_pieces(1)
            run_phase(2, tw[1][:3] + tail_lin_pieces(2), tw[1][3:])
            tw[2] = tail_win_pieces(2)
            run_phase(3, tw[2][:3] + tail_lin_pieces(3), tw[2][3:])
            for p in tail_win_pieces(3):
                p()

    nc.compile()
    return nc


def _prep_inputs(inputs):
    """Host-side sharding/packing. Returns per-core input maps."""
    h = np.asarray(inputs["hidden_states"], np.float32).reshape(S, D)
    ht = np.ascontiguousarray(h.T).astype(BF)

    lin_Wq = np.asarray(inputs["lin_Wq"], np.float32)
    lin_Wk = np.asarray(inputs["lin_Wk"], np.float32)
    lin_Wv = np.asarray(inputs["lin_Wv"], np.float32)
    lin_Wo = np.asarray(inputs["lin_Wo"], np.float32)
    win_Wq = np.asarray(inputs["win_Wq"], np.float32)
    win_Wk = np.asarray(inputs["win_Wk"], np.float32)
    win_Wv = np.asarray(inputs["win_Wv"], np.float32)
    win_Wo = np.asarray(inputs["win_Wo"], np.float32)

    # constant mask tiles
    p = np.arange(128)[:, None]
    f = np.arange(128)[None, :]
    msk = np.zeros((128, 256), np.float32)
    msk[:, 0:128] = (p <= f)          # lin diag mask (t <= s)
    msk[:, 128:256] = (p < f)         # win partial mask (t < s)

    in_maps = []
    for c in range(NCORES):
        a, b = 2 * c, 2 * c + 1
        wqk = np.zeros((D, 384), np.float32)
        wqk[:, 0:64] = win_Wq[:, a * HD : (a + 1) * HD]
        wqk[:, 64:128] = win_Wq[:, b * HD : (b + 1) * HD]
        wqk[:, 128:192] = win_Wk[:, a * HD : (a + 1) * HD]
        wqk[:, 192:256] = win_Wk[:, b * HD : (b + 1) * HD]
        wqk[:, 256:272] = lin_Wq[:, a * FD : (a + 1) * FD] * 0.5
        wqk[:, 288:304] = lin_Wq[:, b * FD : (b + 1) * FD] * 0.5
        wqk[:, 320:336] = lin_Wk[:, a * FD : (a + 1) * FD] * 0.5
        wqk[:, 352:368] = lin_Wk[:, b * FD : (b + 1) * FD] * 0.5
        wv = np.zeros((D, 256), np.float32)
        wv[:, 0:64] = lin_Wv[:, a * HD : (a + 1) * HD] * 0.5
        wv[:, 64:128] = lin_Wv[:, b * HD : (b + 1) * HD] * 0.5
        wv[:, 128:192] = win_Wv[:, a * HD : (a + 1) * HD]
        wv[:, 192:256] = win_Wv[:, b * HD : (b + 1) * HD]
        wo = np.zeros((256, D), np.float32)
        wo[0:64] = lin_Wo[a * HD : (a + 1) * HD]
        wo[64:128] = win_Wo[a * HD : (a + 1) * HD]
        wo[128:192] = lin_Wo[b * HD : (b + 1) * HD]
        wo[192:256] = win_Wo[b * HD : (b + 1) * HD]
        in_maps.append(
            {
                "ht": ht,
                "wqk": wqk.astype(BF),
                "wv": wv.astype(BF),
                "wo": wo.astype(BF),
                "msk": msk.astype(BF),
                "orow": np.ones((1, S), np.float32).astype(BF),
            }
        )
    return in_maps


def kernel(**inputs) -> np.ndarray:
    from concourse.bass_utils import run_bass_kernel_spmd

    if "nc" not in _CACHE:
        _CACHE["nc"] = _build_nc()
    nc = _CACHE["nc"]
    in_maps = _prep_inputs(inputs)
    res = run_bass_kernel_spmd(nc, in_maps, core_ids=list(range(NCORES)))
    out = np.zeros((S, D), np.float32)
    for r in res.results:
        out += r["out"]
    return out.reshape(1, S, D)


if __name__ == "__main__":
    nc = _build_nc()
    print("built ok")


# revision 33
# speedup vs baseline: 1.1182x; 1.0121x over previous
"""Trainium2 Bass kernel for nn_Based_40630390620259 (sparse_attention).

Architecture ("Based"-style): linear (Taylor feature-map) attention +
windowed softmax attention, 16 heads, S=2048, D=1024.

Math identities used (verified against the reference to 1e-6):
  - Taylor feature map inner product collapses:
        qf.kf = 1 + (q.k)/4 + (q.k)^2/32 = 0.5 + 0.5*(1 + q.k/4)^2
    so the 273-dim feature space is never materialized. With Wq,Wk scaled
    by 0.5 on the host and a constant ones-row appended to q/k (K=17
    matmul), the PE produces m'' = 1 + q.k/4 directly; sq = m''^2 on DVE.
  - The 0.5 factor is folded into the V projection weights; the +0.5
    constant term contributes a causal cumulative sum CUM of the
    (0.5-scaled) v rows, computed with 16 N=128 matmuls against an
    upper-triangular ones block + a recursive per-partition scalar-add.
    CUM row 64 (from the 0.5-constant column) equals 0.5*(s+1), which is
    exactly the constant-term part of the reference denominator.
  - win path: scores^T computed as [t,s] tiles; softmax denominator via a
    ones-column in V'; division deferred through the output projection via
    a gpsimd partition_broadcast of the reciprocal row.

Sharding: tensor-parallel over heads, 2 heads per core, 8 cores. Each core
produces a partial [S, D] output (its heads' contribution); the host sums.
Both heads are processed per t-chunk with 2-way row-strip packing (lin at
array rows 0/32, win at rows 0/64) writing the two halves of paired
[128,1024] PSUM tiles, so elementwise ops cover both heads in one
instruction.

Self-contained: only imports concourse/* from the environment.
"""

import numpy as np
import ml_dtypes

S = 2048
D = 1024
H = 16
FD = 16
HD = 64
W = 256
EPS = 1e-9
NCORES = 8

BF = ml_dtypes.bfloat16

_CACHE = {}


def _build_nc(dbg=False):
    import concourse.bass as bass
    import concourse.mybir as mybir
    import concourse.tile as tile
    from concourse import bacc
    from concourse.bass import ts

    f32 = mybir.dt.float32
    bf16 = mybir.dt.bfloat16
    MULT = mybir.AluOpType.mult
    ADD = mybir.AluOpType.add
    Exp = mybir.ActivationFunctionType.Exp
    Square = mybir.ActivationFunctionType.Square

    nc = bacc.Bacc("TRN2", target_bir_lowering=False)

    ht_d = nc.dram_tensor("ht", [D, S], bf16, kind="ExternalInput")
    wqk_d = nc.dram_tensor("wqk", [D, 384], bf16, kind="ExternalInput")
    wv_d = nc.dram_tensor("wv", [D, 256], bf16, kind="ExternalInput")
    wo_d = nc.dram_tensor("wo", [256, D], bf16, kind="ExternalInput")
    msk_d = nc.dram_tensor("msk", [128, 256], bf16, kind="ExternalInput")
    orow_d = nc.dram_tensor("orow", [1, S], bf16, kind="ExternalInput")
    out_d = nc.dram_tensor("out", [S, D], f32, kind="ExternalOutput")
    if dbg:
        dbg_t = {
            "d_qw": nc.dram_tensor("d_qw", [128, S], f32, kind="ExternalOutput"),
            "d_kw": nc.dram_tensor("d_kw", [128, S], f32, kind="ExternalOutput"),
            "d_qkg": nc.dram_tensor("d_qkg", [128, 2 * S], f32, kind="ExternalOutput"),
            "d_v4": nc.dram_tensor("d_v4", [128, 16 * 260], f32, kind="ExternalOutput"),
            "d_cum0": nc.dram_tensor("d_cum0", [65, S], f32, kind="ExternalOutput"),
            "d_ul0": nc.dram_tensor("d_ul0", [65, S], f32, kind="ExternalOutput"),
            "d_uw0": nc.dram_tensor("d_uw0", [65, S], f32, kind="ExternalOutput"),
            "d_den": nc.dram_tensor("d_den", [128, S], f32, kind="ExternalOutput"),
            "d_recf": nc.dram_tensor("d_recf", [128, S], f32, kind="ExternalOutput"),
            "d_scl0": nc.dram_tensor("d_scl0", [128, S], f32, kind="ExternalOutput"),
        }

    NJ = 4          # number of 512-wide s blocks
    SB = 512        # s block width
    NT = 16         # number of 128-wide t chunks

    with tile.TileContext(nc) as tc:
        with (
            tc.tile_pool(name="sb", bufs=1) as sb,
            tc.tile_pool(name="sqp", bufs=5) as sqp,
            tc.tile_pool(name="exp", bufs=5) as exq,
            tc.tile_pool(name="stg", bufs=3) as stg,
            tc.tile_pool(name="bct", bufs=2) as bct,
            tc.tile_pool(name="psA", bufs=2, space="PSUM") as psA,
            tc.tile_pool(name="psAcc", bufs=2, space="PSUM") as psAcc,
            tc.tile_pool(name="psO", bufs=2, space="PSUM") as psO,
        ):
            # ---------------- persistent SBUF tiles ----------------
            ht_sb = sb.tile([128, 8, S], bf16, name="ht_sb")
            wqk_sb = sb.tile([128, 8, 384], bf16, name="wqk_sb")
            wv_sb = sb.tile([128, 8, 256], bf16, name="wv_sb")
            wo_sb = sb.tile([128, 2, 1024], bf16, name="wo_sb")
            msk_sb = sb.tile([128, 256], bf16, name="msk_sb")
            # qkg: lin q/k with ones row; h0 rows 0:17, h1 rows 32:49;
            # free index 0 = q, 1 = k
            qkg_sb = sb.tile([128, 2, S], bf16, name="qkg_sb")
            qw_sb = sb.tile([128, S], bf16, name="qw_sb")
            kw_sb = sb.tile([128, S], bf16, name="kw_sb")
            v4_sb = sb.tile([128, NT, 260], bf16, name="v4_sb")
            cum_sb = [sb.tile([65, S], f32, name=f"cum{h}_sb") for h in range(2)]
            ul_sb = [sb.tile([65, S], f32, name=f"ul{h}_sb") for h in range(2)]
            uw_sb = [sb.tile([65, S], f32, name=f"uw{h}_sb") for h in range(2)]
            scl_sb = [sb.tile([128, S], bf16, name=f"scl{h}_sb") for h in range(2)]
            den_sb = sb.tile([128, S], f32, name="den_sb")
            recf_sb = sb.tile([128, S], f32, name="recf_sb")
            recr_sb = sb.tile([1, 4, SB], f32, name="recr_sb")

            v4r = v4_sb.rearrange("p s (g x) -> p s g x", x=65)

            # ---------------- load inputs ----------------
            for k in range(8):
                nc.sync.dma_start(out=wqk_sb[:, k, :], in_=wqk_d[ts(k, 128), :])
                nc.sync.dma_start(
                    out=ht_sb[:, k, 0:1024], in_=ht_d[ts(k, 128), 0:1024]
                )
            for k in range(8):
                nc.sync.dma_start(
                    out=ht_sb[:, k, 1024:2048], in_=ht_d[ts(k, 128), 1024:2048]
                )
                nc.sync.dma_start(out=wv_sb[:, k, :], in_=wv_d[ts(k, 128), :])
            for k in range(2):
                nc.sync.dma_start(out=wo_sb[:, k, :], in_=wo_d[ts(k, 128), :])
            nc.sync.dma_start(out=msk_sb[:, :], in_=msk_d[:, :])
            # constant columns of v4: 0.5 for lin heads, 1.0 for win heads
            nc.gpsimd.memset(v4r[:, :, 0:2, 64], 0.5)
            nc.gpsimd.memset(v4r[:, :, 2:4, 64], 1.0)
            # garbage rows of den/recf must stay finite and initialized
            nc.gpsimd.memset(den_sb[:, :], 1.0)
            nc.gpsimd.memset(recf_sb[:, :], 1.0)

            # ---------------- phase 1a: q/k projections (paired j) ----------
            # wqk columns: [qw_a(64) qw_b(64) | kw_a(64) kw_b(64) |
            #   qlin_a@256 qlin_b@288 klin_a@320 klin_b@352 (16 each)]
            for jp in range(2):
                for blk in range(3):
                    c0 = blk * 128
                    js2 = ts(jp, 1024)
                    pp = psA.tile([128, 1024], f32, name="pp", tag="psA")
                    for jh in range(2):
                        for k in range(8):
                            nc.tensor.matmul(
                                pp[:, ts(jh, SB)],
                                lhsT=wqk_sb[:, k, c0 : c0 + 128],
                                rhs=ht_sb[:, k, ts(2 * jp + jh, SB)],
                                start=(k == 0),
                                stop=(k == 7),
                            )
                    if blk == 0:
                        nc.scalar.copy(out=qw_sb[:, js2], in_=pp[:, :])
                    elif blk == 1:
                        nc.scalar.copy(out=kw_sb[:, js2], in_=pp[:, :])
                    else:
                        nc.vector.tensor_copy(
                            out=qkg_sb[0:48, 0, js2], in_=pp[0:48, :]
                        )
                        nc.vector.tensor_copy(
                            out=qkg_sb[0:48, 1, js2], in_=pp[64:112, :]
                        )
                        # restore ones rows clobbered by the 48-row copies
                        for qk in range(2):
                            for r in (16, 48):
                                nc.sync.dma_start(
                                    out=qkg_sb[r : r + 1, qk, js2],
                                    in_=orow_d[0:1, js2],
                                )

            # ---------------- phase 1b: v projections (paired st) -----------
            for sp_ in range(8):
                st0 = 2 * sp_
                pv = psA.tile([128, 1024], f32, name="pv", tag="psA")
                for sh in range(2):
                    for k in range(8):
                        nc.tensor.matmul(
                            pv[:, sh * 256 : sh * 256 + 256],
                            lhsT=ht_sb[:, k, ts(st0 + sh, 128)],
                            rhs=wv_sb[:, k, :],
                            start=(k == 0),
                            stop=(k == 7),
                        )
                nc.vector.tensor_copy(
                    out=v4r[:, st0 : st0 + 2, :, 0:64],
                    in_=pv[:, 0:512].rearrange("p (s g x) -> p s g x", s=2, x=64),
                )

            # ---------------- phase 1c: CUM (cumulative v-half sums) -------
            # cum[h][d, s] = sum_{t<=s} vlin_half[t, d]; row 64 = 0.5*(s+1)
            for h in range(2):
                lin_sl = slice(65 * h, 65 * h + 65)
                for sj in range(NT):
                    icp = psA.tile([128, 1024], f32, name="icp", tag="psA")
                    nc.tensor.matmul(
                        icp[0:65, 0:128],
                        lhsT=v4_sb[:, sj, lin_sl],
                        rhs=msk_sb[:, 0:128],
                        start=True,
                        stop=True,
                    )
                    if sj == 0:
                        nc.vector.tensor_scalar(
                            out=cum_sb[h][:, 0:128],
                            in0=icp[0:65, 0:128],
                            scalar1=0.0,
                            scalar2=None,
                            op0=ADD,
                        )
                    else:
                        nc.vector.tensor_scalar(
                            out=cum_sb[h][:, ts(sj, 128)],
                            in0=icp[0:65, 0:128],
                            scalar1=cum_sb[h][:, sj * 128 - 1 : sj * 128],
                            scalar2=None,
                            op0=ADD,
                        )

            # ---------------- phase 2 + pipelined tails ----------------
            # Emitted as interleaved blocks: lin/win alternate per t-chunk,
            # accumulation matmuls trail their producer by 2 blocks, and the
            # previous j-block's tail pieces are spliced into the stream so
            # no engine queues long dependency stalls.
            def phase2_blocks(j):
                js = ts(j, SB)
                lim_l = 4 * j + 4
                lim_w = min(16, 4 * j + 6)
                st_ = {}

                def begin():
                    st_["qkv"] = [
                        psAcc.tile([65, SB], f32, name=f"qkv{h}", tag="acc")
                        for h in range(2)
                    ]
                    st_["sq"] = {}

                def lin_blk(ti):
                    mp = psA.tile([128, 1024], f32, name="mp", tag="psA")
                    for h, (p0, p1) in enumerate(((0, 17), (32, 49))):
                        nc.tensor.matmul(
                            mp[:, ts(h, SB)],
                            lhsT=qkg_sb[p0:p1, 1, ts(ti, 128)],
                            rhs=qkg_sb[p0:p1, 0, js],
                            start=True,
                            stop=True,
                        )
                    sq = sqp.tile([128, 1024], bf16, name="sq", tag="sq")
                    col0 = max(0, ti - 4 * j) * 128
                    sqr = sq.rearrange("p (g x) -> p g x", x=SB)
                    mpr = mp.rearrange("p (g x) -> p g x", x=SB)
                    if col0:
                        nc.gpsimd.memset(sqr[:, :, 0:col0], 0.0)
                    nc.scalar.activation(
                        sqr[:, :, col0:SB], mpr[:, :, col0:SB], Square
                    )
                    sd = ti - 4 * j
                    if 0 <= sd <= 3:
                        for h in range(2):
                            dsl = slice(h * SB + sd * 128, h * SB + (sd + 1) * 128)
                            nc.gpsimd.tensor_tensor(
                                out=sq[:, dsl], in0=sq[:, dsl],
                                in1=msk_sb[:, 0:128], op=MULT,
                            )
                    st_["sq"][ti] = sq

                def lin_acc(ti):
                    sqt = st_["sq"].pop(ti)
                    for h in range(2):
                        nc.tensor.matmul(
                            st_["qkv"][h][:, :],
                            lhsT=v4_sb[:, ti, slice(65 * h, 65 * h + 65)],
                            rhs=sqt[:, ts(h, SB)],
                            start=(ti == 0),
                            stop=(ti == lim_l - 1),
                            skip_group_check=True,
                        )

                def lin_end():
                    for h in range(2):
                        nc.vector.tensor_tensor(
                            out=ul_sb[h][:, js],
                            in0=st_["qkv"][h][0:65, :],
                            in1=cum_sb[h][:, js],
                            op=ADD,
                        )
                        nc.sync.dma_start(
                            out=den_sb[32 * h : 32 * h + 1, js],
                            in_=ul_sb[h][64:65, js],
                        )

                def win_begin():
                    st_["nt"] = [
                        psAcc.tile([65, SB], f32, name=f"nt{h}", tag="acc")
                        for h in range(2)
                    ]
                    st_["ex"] = {}

                def win_blk(ti):
                    sp = psA.tile([128, 1024], f32, name="sp", tag="psA")
                    for h in range(2):
                        hsl = slice(64 * h, 64 * h + 64)
                        nc.tensor.matmul(
                            sp[:, ts(h, SB)],
                            lhsT=kw_sb[hsl, ts(ti, 128)],
                            rhs=qw_sb[hsl, js],
                            start=True,
                            stop=True,
                        )
                    ex = exq.tile([128, 1024], bf16, name="ex", tag="ex")
                    col0 = max(0, ti - 2 - 4 * j) * 128
                    exr = ex.rearrange("p (g x) -> p g x", x=SB)
                    spr = sp.rearrange("p (g x) -> p g x", x=SB)
                    if col0:
                        nc.gpsimd.memset(exr[:, :, 0:col0], 0.0)
                    nc.scalar.activation(
                        exr[:, :, col0:SB], spr[:, :, col0:SB], Exp, scale=0.125
                    )
                    sd = ti - 2 - 4 * j
                    if 0 <= sd <= 3:
                        for h in range(2):
                            dsl = slice(h * SB + sd * 128, h * SB + (sd + 1) * 128)
                            nc.gpsimd.tensor_tensor(
                                out=ex[:, dsl], in0=ex[:, dsl],
                                in1=msk_sb[:, 128:256], op=MULT,
                            )
                    st_["ex"][ti] = ex

                def win_acc(ti):
                    ext = st_["ex"].pop(ti)
                    for h in range(2):
                        nc.tensor.matmul(
                            st_["nt"][h][:, :],
                            lhsT=v4_sb[:, ti, slice(130 + 65 * h, 195 + 65 * h)],
                            rhs=ext[:, ts(h, SB)],
                            start=(ti == 0),
                            stop=(ti == lim_w - 1),
                            skip_group_check=True,
                        )

                def win_end():
                    for h in range(2):
                        nc.vector.tensor_copy(
                            out=uw_sb[h][:, js], in_=st_["nt"][h][0:65, :]
                        )
                        nc.sync.dma_start(
                            out=den_sb[64 + 32 * h : 65 + 32 * h, js],
                            in_=uw_sb[h][64:65, js],
                        )

                # block list: lin chain then win chain, lag-2 accumulation
                blocks = [begin]
                for ti in range(lim_l):
                    blocks.append(lambda ti=ti: lin_blk(ti))
                    if ti >= 2:
                        blocks.append(lambda ti=ti - 2: lin_acc(ti))
                blocks.append(lambda: lin_acc(lim_l - 2))
                blocks.append(lambda: lin_acc(lim_l - 1))
                blocks.append(lin_end)
                blocks.append("WIN_START")
                blocks.append(win_begin)
                for ti in range(lim_w):
                    blocks.append(lambda ti=ti: win_blk(ti))
                    if ti >= 2:
                        blocks.append(lambda ti=ti - 2: win_acc(ti))
                blocks.append("FLUSH_START")
                blocks.append(lambda: win_acc(lim_w - 2))
                blocks.append(lambda: win_acc(lim_w - 1))
                blocks.append(win_end)
                return blocks

            recf4v = None

            def tail_lin_pieces(j):
                js = ts(j, SB)
                pieces = []

                def lin_recip():
                    nc.vector.tensor_scalar(
                        out=recf_sb[0:33, js], in0=den_sb[0:33, js],
                        scalar1=EPS, scalar2=None, op0=ADD,
                    )
                    nc.vector.reciprocal(
                        out=recf_sb[0:33, js], in_=recf_sb[0:33, js]
                    )
                    rv = recf_sb.rearrange("(a b) f -> a b f", b=32)
                    nc.sync.dma_start(
                        out=recr_sb[0:1, 0:2, :], in_=rv[0:2, 0, js]
                    )

                pieces.append(lin_recip)

                def scale(h):
                    bc = bct.tile([64, SB], f32, name="bc", tag="bc")
                    nc.gpsimd.partition_broadcast(bc[:, :], recr_sb[0:1, h, :])
                    nc.vector.tensor_tensor(
                        out=scl_sb[h][0:64, js],
                        in0=ul_sb[h][0:64, js],
                        in1=bc[:, :],
                        op=MULT,
                    )

                for h in range(2):
                    pieces.append(lambda h=h: scale(h))
                return pieces

            def tail_win_pieces(j):
                js = ts(j, SB)
                pieces = []

                def win_recip():
                    nc.vector.tensor_scalar(
                        out=recf_sb[64:97, js], in0=den_sb[64:97, js],
                        scalar1=EPS, scalar2=None, op0=ADD,
                    )
                    nc.vector.reciprocal(
                        out=recf_sb[64:97, js], in_=recf_sb[64:97, js]
                    )
                    rv = recf_sb.rearrange("(a b) f -> a b f", b=32)
                    nc.sync.dma_start(
                        out=recr_sb[0:1, 2:4, :], in_=rv[2:4, 0, js]
                    )

                pieces.append(win_recip)

                def scale(h):
                    bc = bct.tile([64, SB], f32, name="bc", tag="bc")
                    nc.gpsimd.partition_broadcast(bc[:, :], recr_sb[0:1, 2 + h, :])
                    nc.vector.tensor_tensor(
                        out=scl_sb[h][64:128, js],
                        in0=uw_sb[h][0:64, js],
                        in1=bc[:, :],
                        op=MULT,
                    )

                for h in range(2):
                    pieces.append(lambda h=h: scale(h))

                def final(st, nb):
                    po = psO.tile([128, SB], f32, name="po", tag="po")
                    for h in range(2):
                        nc.tensor.matmul(
                            po[:, :],
                            lhsT=scl_sb[h][:, ts(st, 128)],
                            rhs=wo_sb[:, h, ts(nb, SB)],
                            start=(h == 0),
                            stop=(h == 1),
                            skip_group_check=True,
                        )
                    so = stg.tile([128, SB], f32, name="so", tag="so")
                    if (st + nb) % 2 == 0:
                        nc.vector.tensor_copy(out=so[:, :], in_=po[:, :])
                    else:
                        nc.scalar.copy(out=so[:, :], in_=po[:, :])
                    nc.sync.dma_start(
                        out=out_d[ts(st, 128), ts(nb, SB)], in_=so[:, :]
                    )

                for st in range(4 * j, 4 * j + 4):
                    for nb in range(2):
                        pieces.append(lambda st=st, nb=nb: final(st, nb))
                return pieces

            def interleave(blocks, pieces):
                wstart = blocks.index("WIN_START") if "WIN_START" in blocks else 0
                blocks = [b for b in blocks if b != "WIN_START"]
                if not pieces:
                    for b in blocks:
                        b()
                    return
                nwin = len(blocks) - wstart
                stride = max(1, nwin // (len(pieces) + 1))
                pi = 0
                for i, b in enumerate(blocks):
                    b()
                    if i >= wstart and (i - wstart + 1) % stride == 0 and pi < len(pieces):
                        pieces[pi]()
                        pi += 1
                while pi < len(pieces):
                    pieces[pi]()
                    pi += 1

            def run_phase(j, early, late):
                blocks = phase2_blocks(j)
                wstart = blocks.index("WIN_START")
                blocks.remove("WIN_START")
                fstart = blocks.index("FLUSH_START")
                blocks.remove("FLUSH_START")
                nwin = fstart - wstart
                stride = max(1, nwin // (len(early) + 1)) if early else 10**9
                pi = 0
                for i, b in enumerate(blocks):
                    if i == fstart:
                        # fill the ACT-backlog stall at the accumulation
                        # flush with independent PE work (previous finals)
                        while pi < len(early):
                            early[pi]()
                            pi += 1
                        for p in late:
                            p()
                    b()
                    if i >= wstart and (i - wstart + 1) % stride == 0 and pi < len(early):
                        early[pi]()
                        pi += 1
                while pi < len(early):
                    early[pi]()
                    pi += 1

            tw = {}
            run_phase(0, tail_lin_pieces(0), [])
            tw[0] = tail_win_pieces(0)
            run_phase(1, tw[0][:3] + tail_lin_pieces(1), tw[0][3:])
            tw[1] = tail_win_pieces(1)
            run_phase(2, tw[1][:3] + tail_lin_pieces(2), tw[1][3:])
            tw[2] = tail_win_pieces(2)
            run_phase(3, tw[2][:3] + tail_lin_pieces(3), tw[2][3:])
            for p in tail_win_pieces(3):
                p()

    nc.compile()
    return nc


def _prep_inputs(inputs):
    """Host-side sharding/packing. Returns per-core input maps."""
    h = np.asarray(inputs["hidden_states"], np.float32).reshape(S, D)
    ht = np.ascontiguousarray(h.T).astype(BF)

    lin_Wq = np.asarray(inputs["lin_Wq"], np.float32)
    lin_Wk = np.asarray(inputs["lin_Wk"], np.float32)
    lin_Wv = np.asarray(inputs["lin_Wv"], np.float32)
    lin_Wo = np.asarray(inputs["lin_Wo"], np.float32)
    win_Wq = np.asarray(inputs["win_Wq"], np.float32)
    win_Wk = np.asarray(inputs["win_Wk"], np.float32)
    win_Wv = np.asarray(inputs["win_Wv"], np.float32)
    win_Wo = np.asarray(inputs["win_Wo"], np.float32)

    # constant mask tiles
    p = np.arange(128)[:, None]
    f = np.arange(128)[None, :]
    msk = np.zeros((128, 256), np.float32)
    msk[:, 0:128] = (p <= f)          # lin diag mask (t <= s)
    msk[:, 128:256] = (p < f)         # win partial mask (t < s)

    in_maps = []
    for c in range(NCORES):
        a, b = 2 * c, 2 * c + 1
        wqk = np.zeros((D, 384), np.float32)
        wqk[:, 0:64] = win_Wq[:, a * HD : (a + 1) * HD]
        wqk[:, 64:128] = win_Wq[:, b * HD : (b + 1) * HD]
        wqk[:, 128:192] = win_Wk[:, a * HD : (a + 1) * HD]
        wqk[:, 192:256] = win_Wk[:, b * HD : (b + 1) * HD]
        wqk[:, 256:272] = lin_Wq[:, a * FD : (a + 1) * FD] * 0.5
        wqk[:, 288:304] = lin_Wq[:, b * FD : (b + 1) * FD] * 0.5
        wqk[:, 320:336] = lin_Wk[:, a * FD : (a + 1) * FD] * 0.5
        wqk[:, 352:368] = lin_Wk[:, b * FD : (b + 1) * FD] * 0.5
        wv = np.zeros((D, 256), np.float32)
        wv[:, 0:64] = lin_Wv[:, a * HD : (a + 1) * HD] * 0.5
        wv[:, 64:128] = lin_Wv[:, b * HD : (b + 1) * HD] * 0.5
        wv[:, 128:192] = win_Wv[:, a * HD : (a + 1) * HD]
        wv[:, 192:256] = win_Wv[:, b * HD : (b + 1) * HD]
        wo = np.zeros((256, D), np.float32)
        wo[0:64] = lin_Wo[a * HD : (a + 1) * HD]
        wo[64:128] = win_Wo[a * HD : (a + 1) * HD]
        wo[128:192] = lin_Wo[b * HD : (b + 1) * HD]
        wo[192:256] = win_Wo[b * HD : (b + 1) * HD]
        in_maps.append(
            {
                "ht": ht,
                "wqk": wqk.astype(BF),
                "wv": wv.astype(BF),
                "wo": wo.astype(BF),
                "msk": msk.astype(BF),
                "orow": np.ones((1, S), np.float32).astype(BF),
            }
        )
    return in_maps


def kernel(**inputs) -> np.ndarray:
    from concourse.bass_utils import run_bass_kernel_spmd

    if "nc" not in _CACHE:
        _CACHE["nc"] = _build_nc()
    nc = _CACHE["nc"]
    in_maps = _prep_inputs(inputs)
    res = run_bass_kernel_spmd(nc, in_maps, core_ids=list(range(NCORES)))
    out = np.zeros((S, D), np.float32)
    for r in res.results:
        out += r["out"]
    return out.reshape(1, S, D)


if __name__ == "__main__":
    nc = _build_nc()
    print("built ok")
